# revision 38
# baseline (speedup 1.0000x reference)
"""Trainium2 Bass kernel for the attention-based encoder.

Computation (per batch b):
    a      = P @ y[b]                                  # [D]
    logits = x[b] @ a                                  # [M]
    p_un   = exp(logits - 16)                          # fixed shift (cancels)
    Z      = sum(p_un)
    W[t]   = p_un[t-1] + p_un[t] + p_un[t+1] + p_un[t+2]  (zero-padded), W[M-1] = 0
    enc[b] = (W @ x[b]) / (Q * Z)                      # [D]

Design (each point validated by HW ablation / TimelineSim analysis):
  * all HBM traffic in bf16 (x 16 MiB + P 10 MiB per core; rel err 3.3e-3
    vs the 2e-2 gate) - halves the DMA floor and runs every matmul at
    bf16 rate instead of multi-pass fp32.
  * host pre-arranges x and P^T so every DMA is one large transfer with
    contiguous 16-80 KiB per partition, and pads the per-partition row
    by 2176 B: the power-of-2 HBM stride otherwise aliases banks
    (187 -> 315 GB/s measured).
  * logits via ONE fused op per m-tile: DVE scalar_tensor_tensor
    (out=(x*1)*a_rep, accum_out=full-1024 dot) — n_pool tiles per batch
    instead use Pool tensor_mul + ACT Copy-accumulate, balancing the
    three element-wise engines.  (tensor_tensor_reduce and Pool-side STT
    both fail walrus codegen "ISA wrong length" — only DVE STT works.)
  * per-batch split-half pipeline: tiles 0-7 logits -> exp(+Z accum) ->
    W cols 0..6 -> PE tail t0..6 overlap tiles 8-15 logits; W = 4-tap
    window of p via banded matmuls with +-2 halo columns.
  * x halves DMAed in 2 pieces (last batch's h1 in 4) so compute chases
    the bus; last P chunk's DMA split 7+1 k-tiles so phase A trails the
    29-us P transfer by <1 us.
  * a replicated across partitions with a selector matmul; copies ride
    the idle ACT engine, scheduled b0/b1 in phase A and b+2 inside
    batch b (keeps the single 'pr' PSUM bank ring short without the PE
    tail ever gating a batch start).
  * the normalizer chain is deferred: Q folded into the Z ones-matmul
    weights, reciprocal (DVE) + PSUM->enc scales (ACT, dep on PE tail
    stop) of batch b emitted at batch b+2 so the in-order DVE/ACT
    queues never stall between batches.
  * reps>1 builds unroll the For_i body by 8 (HW: 126.9/122.9/120.4 us
    per rep at unroll 2/4/8).  Flat (no-loop) unrolling collapses to
    1.7 ms/rep on HW (instruction-memory thrash) — loops are mandatory.
    Emitting the output DMA from the ACT DGE queue instead of SP was
    23 us/rep slower — keep every DMA on nc.sync.
  * pshard=True phase A (P split over cores + AllReduce) works at
    reps=1 and is equally accurate (3.298e-3), but collectives fail at
    runtime inside a For_i reps-loop, so it stays off.

Sharding: data-parallel over batch, 4 batches per core on 8 cores.
"""

import numpy as np
import ml_dtypes

import concourse.bass as bass
import concourse.mybir as mybir
from concourse.tile import TileContext
from concourse.bass_utils import run_bass_kernel_spmd

# ---------------------------------------------------------------------------
# This container's walrus supports only ONE sync wait per instruction ("Too
# many sync wait commands" at codegen otherwise), while Tile freely attaches
# several.  Post-pass: hoist excess waits onto injected same-engine NoOps
# placed immediately before the over-subscribed instruction.
# ---------------------------------------------------------------------------

_MAX_WAITS = 1


def split_sync_waits(nc: bass.Bass) -> None:
    uid = 0
    for fn in nc.m.functions:
        for blk in fn.blocks:
            new_insts = []
            for inst in blk.instructions:
                si = inst.sync_info
                waits = list(si.on_wait) if si and si.on_wait else []
                if len(waits) > _MAX_WAITS:
                    for w in waits[:-_MAX_WAITS]:
                        uid += 1
                        ev = mybir.InstEventSemaphore(
                            name=f"{inst.name}_hw{uid}",
                            opcode="EventSemaphore",
                            ins=[],
                            outs=[],
                            sync_info=mybir.SyncInfo(on_wait=[w], on_update=[]),
                        )
                        ev.engine = inst.engine
                        new_insts.append(ev)
                    si.on_wait = waits[-_MAX_WAITS:]
                new_insts.append(inst)
            blk.instructions[:] = new_insts

# ---------------------------------------------------------------------------

B, M, D, CD = 32, 2048, 1024, 5120
Q = 2
NCORES = 8
BPC = B // NCORES          # batches per core
NT = M // 128              # m-tiles per batch
KT = CD // 128             # k-tiles of the P contraction
KCH = 8                    # k-tiles per P DMA chunk
NCH = KT // KCH
KTC = KT // NCORES  # k-tiles per core when P is sharded
PAD = 1088                 # bf16 elems of per-partition-row padding: breaks
                           # the power-of-2 HBM stride aliasing (187 -> 315 GB/s)
HNT = NT // 2              # m-tiles per xb half-load
F32 = mybir.dt.float32
BF16 = mybir.dt.bfloat16
ALU = mybir.AluOpType
AFT = mybir.ActivationFunctionType


def build_nc(reps: int = 1, n_batches: int = BPC, do_phase_b: bool = True,
             stop_after: str = "full", skip: tuple = (),
             logits_mode: str = "split", xhalves: bool = True,
             n_dve: int = 16, n_gps: int = 0,
             accum_dump: str = "inplace", fuse_g: int = 4,
             n_actg: int = 0, pshard: bool = False,
             unroll: int = 2, n_pool: int = 0, xpieces: int = 2,
             xbufs: int = 3, ptbufs: int = 3, scrbufs: int = 3) -> bass.Bass:
    nc = bass.Bass(num_devices=NCORES) if pshard else bass.Bass()
    xs = nc.declare_dram_parameter("xs", [128, BPC, NT * D + PAD], BF16, isOutput=False)
    if pshard:
        # P sharded over cores: each core holds KTC k-tiles of P^T, the y
        # slices matching those k-tiles for ALL B batches, and a selector
        # that picks this core's batch rows out of the allreduced aT.
        pt = nc.declare_dram_parameter("pt", [128, KTC * D + PAD], BF16,
                                       isOutput=False)
        ys = nc.declare_dram_parameter("ys", [128, KTC, B], BF16,
                                       isOutput=False)
        sel = nc.declare_dram_parameter("sel", [B, BPC * 128], BF16,
                                        isOutput=False)
        cc_in = nc.dram_tensor("cc_in", [B, D], F32, addr_space="Local")
        cc_out = nc.dram_tensor("cc_out", [B, D], F32, addr_space="Local")
    else:
        pt = nc.declare_dram_parameter("pt", [128, KT * D + PAD], BF16,
                                       isOutput=False)
        ys = nc.declare_dram_parameter("ys", [128, KT, BPC], BF16,
                                       isOutput=False)
    enc = nc.declare_dram_parameter("enc", [1, BPC * D], F32, isOutput=True)

    with TileContext(nc) as tc:
        with (
            tc.tile_pool(name="const", bufs=1) as const_pool,
            tc.tile_pool(name="ysp", bufs=1) as ys_pool,
            tc.tile_pool(name="ptp", bufs=ptbufs) as pt_pool,
            tc.tile_pool(name="xp", bufs=xbufs) as x_pool,
            tc.tile_pool(name="arep", bufs=1) as arep_pool,
            tc.tile_pool(name="small", bufs=1) as small_pool,
            tc.tile_pool(name="tiny", bufs=2) as tiny_pool,
            tc.tile_pool(name="scr", bufs=scrbufs) as scr_pool,
            tc.tile_pool(name="ps", bufs=1, space="PSUM") as psum_pool,
            tc.tile_pool(name="pse", bufs=2, space="PSUM") as psum_e_pool,
        ):
            ones_col = const_pool.tile([128, 1], F32)
            nc.vector.memset(ones_col[:], 1.0)
            # Q-scaled ones: folds the x_bar 1/Q into Z at no cost
            q_col = const_pool.tile([128, 1], F32)
            nc.vector.memset(q_col[:], float(Q))
            nshift = const_pool.tile([128, 1], F32)
            nc.vector.memset(nshift[:], -16.0)
            if pshard:
                ys_sb = const_pool.tile([128, KTC, B], BF16)
                nc.sync.dma_start(out=ys_sb[:], in_=ys[:])
                sel_sb = const_pool.tile([B, BPC * 128], BF16)
                nc.sync.dma_start(out=sel_sb[:], in_=sel[:])
            else:
                ys_sb = const_pool.tile([128, KT, BPC], BF16)
                nc.sync.dma_start(out=ys_sb[:], in_=ys[:])

            # banded matrices for the 4-tap sliding-window sum W = S4 @ p:
            # s4[c, f] = 1 iff f - c in {-2, -1, 0, 1}; corner matrices
            # carry the +-2-element inter-tile halo.
            s4 = const_pool.tile([128, 128], F32)
            nc.gpsimd.memset(s4[:], 0.0)
            for base in (1, 0, -1, -2):
                nc.gpsimd.affine_select(
                    out=s4[:], in_=s4[:], compare_op=ALU.not_equal, fill=1.0,
                    base=base, pattern=[[-1, 128]], channel_multiplier=1,
                )
            sprev = const_pool.tile([128, 128], F32)
            nc.gpsimd.memset(sprev[:], 0.0)
            nc.gpsimd.affine_select(
                out=sprev[:], in_=sprev[:], compare_op=ALU.not_equal, fill=1.0,
                base=-127, pattern=[[-1, 128]], channel_multiplier=1,
            )
            snext = const_pool.tile([128, 128], F32)
            nc.gpsimd.memset(snext[:], 0.0)
            for base in (126, 127):
                nc.gpsimd.affine_select(
                    out=snext[:], in_=snext[:], compare_op=ALU.not_equal, fill=1.0,
                    base=base, pattern=[[-1, 128]], channel_multiplier=1,
                )

            # last-tile variant of s4 with column M-1 zeroed (W[M-1] = 0)
            s4last = const_pool.tile([128, 128], F32)
            nc.gpsimd.memset(s4last[:], 0.0)
            for base in (1, 0, -1, -2):
                nc.gpsimd.affine_select(
                    out=s4last[:], in_=s4last[:], compare_op=ALU.not_equal,
                    fill=1.0, base=base, pattern=[[-1, 128]],
                    channel_multiplier=1,
                )
            nc.gpsimd.affine_select(
                out=s4last[:], in_=s4last[:], compare_op=ALU.not_equal,
                fill=0.0, base=-127, pattern=[[1, 128]], channel_multiplier=0,
            )

            # selector rows: selb[b][k, m] = 1 iff k == b (4 partitions);
            # a_rep[b] = selb[b].T @ aT broadcasts batch b's row of aT to
            # all 128 partitions without an SBUF round-trip.
            selb = []
            for b in range(BPC):
                sb = const_pool.tile([BPC, 128], BF16, name=f"selb{b}")
                nc.gpsimd.memset(sb[:], 0.0)
                nc.gpsimd.affine_select(
                    out=sb[:], in_=sb[:], compare_op=ALU.not_equal, fill=1.0,
                    base=-b, pattern=[[0, 128]], channel_multiplier=1,
                )
                selb.append(sb)

            a_rep = [
                arep_pool.tile([128, D], BF16, tag=f"a_rep{b}", name=f"a_rep{b}")
                for b in range(BPC)
            ]

            def body(_=None):
                if "phase_a" in skip:
                    [nc.vector.memset(ar[:], 0.001) for ar in a_rep]
                    return body_b()

                if pshard:
                    return phase_a_sharded()

                # ---- Phase A: aT[b, d] = sum_k y[b, k] * PT[k, d] ----
                pa0 = psum_pool.tile([BPC, 512], F32, tag="pa0")
                pa1 = psum_pool.tile([BPC, 512], F32, tag="pa1")
                for c in range(NCH):
                    ptc = pt_pool.tile([128, KCH * D], BF16, tag="ptc")
                    if c == NCH - 1:
                        # split the last chunk's DMA so its final k-tile (and
                        # the phase-A tail) lags the P transfer by ~0.8us
                        # instead of a full 2MiB chunk
                        nc.sync.dma_start(
                            out=ptc[:, 0:(KCH - 1) * D],
                            in_=pt[:, c * KCH * D:(c * KCH + KCH - 1) * D],
                        )
                        nc.sync.dma_start(
                            out=ptc[:, (KCH - 1) * D:KCH * D],
                            in_=pt[:, (c * KCH + KCH - 1) * D:(c + 1) * KCH * D],
                        )
                    else:
                        nc.sync.dma_start(
                            out=ptc[:], in_=pt[:, c * KCH * D:(c + 1) * KCH * D]
                        )
                    for u in range(KCH):
                        t = c * KCH + u
                        nc.tensor.matmul(
                            pa0[:], lhsT=ys_sb[:, t, :],
                            rhs=ptc[:, u * D:u * D + 512],
                            start=(t == 0), stop=(t == KT - 1),
                        )
                        nc.tensor.matmul(
                            pa1[:], lhsT=ys_sb[:, t, :],
                            rhs=ptc[:, u * D + 512:(u + 1) * D],
                            start=(t == 0), stop=(t == KT - 1),
                        )
                aT_sb = small_pool.tile([BPC, D], BF16, tag="aT")
                nc.scalar.copy(out=aT_sb[:, 0:512], in_=pa0[:])
                nc.scalar.copy(out=aT_sb[:, 512:1024], in_=pa1[:])

                if logits_mode == "ttr":
                    # replicate a for the first two batches now (copies on
                    # the mostly-idle ACT engine); b+2's replication is
                    # emitted inside batch b so the PSUM 'pr' ring stays
                    # short while batch starts never wait on the PE tail
                    for b in range(min(2, n_batches if do_phase_b else BPC)):
                        replicate_a(b, aT_sb, eng="act")
                else:
                    for b in range(BPC):
                        replicate_a(b, aT_sb, eng="dve")

                if not do_phase_b:
                    for b in range(BPC):
                        if logits_mode == "ttr" and b >= 2:
                            replicate_a(b, aT_sb, eng="act")
                        nc.gpsimd.dma_start(
                            out=enc[0, b * D:b * D + 512],
                            in_=a_rep[b][0:1, 0:512],
                        )
                    return
                return body_b(aT_sb)

            def replicate_a(b, aT_sb, eng):
                for dh in range(2):
                    pr = psum_pool.tile([128, 512], F32, tag="pr")
                    nc.tensor.matmul(
                        pr[:], lhsT=selb[b][:],
                        rhs=aT_sb[:, dh * 512:(dh + 1) * 512],
                        start=True, stop=True,
                    )
                    if eng == "act":
                        nc.scalar.copy(
                            out=a_rep[b][:, dh * 512:(dh + 1) * 512],
                            in_=pr[:],
                        )
                    else:
                        nc.vector.tensor_copy(
                            a_rep[b][:, dh * 512:(dh + 1) * 512], pr[:]
                        )

            def phase_a_sharded():
                # partial aT for ALL batches from this core's P k-tiles,
                # then an 8-core AllReduce combines the partials.
                pa0 = psum_pool.tile([B, 512], F32, tag="pa0")
                pa1 = psum_pool.tile([B, 512], F32, tag="pa1")
                ptc = pt_pool.tile([128, KTC * D], BF16, tag="ptc")
                nc.sync.dma_start(out=ptc[:], in_=pt[:, 0:KTC * D])
                for kt in range(KTC):
                    nc.tensor.matmul(
                        pa0[:], lhsT=ys_sb[:, kt, :],
                        rhs=ptc[:, kt * D:kt * D + 512],
                        start=(kt == 0), stop=(kt == KTC - 1),
                    )
                    nc.tensor.matmul(
                        pa1[:], lhsT=ys_sb[:, kt, :],
                        rhs=ptc[:, kt * D + 512:(kt + 1) * D],
                        start=(kt == 0), stop=(kt == KTC - 1),
                    )
                aT_part = small_pool.tile([B, D], F32, tag="aT_part")
                nc.vector.tensor_copy(aT_part[:, 0:512], pa0[:])
                nc.vector.tensor_copy(aT_part[:, 512:1024], pa1[:])
                nc.sync.dma_start(out=cc_in[:], in_=aT_part[:])
                nc.gpsimd.collective_compute(
                    "AllReduce", ALU.add,
                    replica_groups=[list(range(NCORES))],
                    ins=[cc_in[:]], outs=[cc_out[:]],
                )
                aT_f32 = small_pool.tile([B, D], F32, tag="aT_f32")
                nc.sync.dma_start(out=aT_f32[:], in_=cc_out[:])
                aT_sb = small_pool.tile([B, D], BF16, tag="aT")
                nc.vector.tensor_copy(aT_sb[:], aT_f32[:])

                for b in range(BPC):
                    for dh in range(2):
                        pr = psum_pool.tile([128, 512], F32, tag="pr")
                        nc.tensor.matmul(
                            pr[:], lhsT=sel_sb[:, b * 128:(b + 1) * 128],
                            rhs=aT_sb[:, dh * 512:(dh + 1) * 512],
                            start=True, stop=True,
                        )
                        nc.vector.tensor_copy(
                            a_rep[b][:, dh * 512:(dh + 1) * 512], pr[:]
                        )
                if not do_phase_b:
                    for b in range(BPC):
                        nc.gpsimd.dma_start(
                            out=enc[0, b * D:b * D + 512],
                            in_=a_rep[b][0:1, 0:512],
                        )
                    return
                return body_b()

            def ttr_batch(b, xh, enc_sb):
                # fused logits: one tensor_tensor_reduce (DVE) or
                # scalar_tensor_tensor (Pool) per m-tile — accum_out takes
                # the full 1024-wide dot against a_rep.  The batch is
                # processed in two m-halves so exp/W/tail of half 0 overlap
                # the second half's logits ops (cuts the per-batch drain).
                logits = tiny_pool.tile([128, NT], F32, tag="logits_a")
                p_pad = tiny_pool.tile([128, NT + 2], F32, tag="p_pad")
                w_pm = tiny_pool.tile([128, NT], BF16, tag="w_pm")
                zc = [tiny_pool.tile([128, 1], F32, tag=f"zc{h}",
                                     name=f"zc{h}") for h in range(2)]
                if b < 2:
                    # pad columns are zeroed once per ring buffer (bufs=2);
                    # later batches reuse the already-zeroed slots
                    nc.vector.memset(p_pad[:, 0:1], 0.0)
                    nc.vector.memset(p_pad[:, NT + 1:NT + 2], 0.0)
                w_ps = psum_pool.tile([128, NT], F32, tag="w_ps")
                pe0 = psum_e_pool.tile([1, 512], F32, tag="pe0")
                pe1 = psum_e_pool.tile([1, 512], F32, tag="pe1")

                for h in range(2):
                    t0, t1 = h * HNT, h * HNT + HNT
                    npool_h = (n_pool + (1 - h)) // 2
                    for t in range(t0, t1):
                        xt = xh[h][:, (t - t0) * D:(t - t0 + 1) * D]
                        if t - t0 < npool_h:
                            # Pool mul + ACT accumulate (Pool cannot encode
                            # the fused STT-with-accum; TTR is unsupported by
                            # this walrus build entirely)
                            scr = scr_pool.tile([128, D], BF16, tag="scr_p")
                            nc.gpsimd.tensor_mul(scr[:], xt, a_rep[b][:])
                            nc.scalar.activation(
                                out=scr[:], in_=scr[:], func=AFT.Copy,
                                accum_out=logits[:, t:t + 1],
                            )
                        else:
                            # fused mul+sum in one DVE op:
                            # out=(x·1)·a, accum_out=Σ_free out
                            scr = scr_pool.tile([128, D], BF16, tag="scr_v")
                            nc.vector.scalar_tensor_tensor(
                                out=scr[:], in0=xt, scalar=1.0,
                                in1=a_rep[b][:], op0=ALU.mult, op1=ALU.mult,
                                accum_out=logits[:, t:t + 1],
                            )

                    # softmax numerator for this half (fixed shift cancels);
                    # Z partial lands in zc[h] via the ACT accumulator
                    nc.scalar.activation(
                        out=p_pad[:, 1 + t0:1 + t1], in_=logits[:, t0:t1],
                        func=AFT.Exp, bias=nshift[:], scale=1.0,
                        accum_out=zc[h][:],
                    )

                    # W columns + PE tail for this half (halo: W col c needs
                    # p_pad cols c..c+2, all present once this half's exp is
                    # done)
                    if h == 0:
                        nc.tensor.matmul(w_ps[:, 0:HNT - 1], lhsT=s4[:],
                                         rhs=p_pad[:, 1:HNT], start=True,
                                         stop=False)
                        nc.tensor.matmul(w_ps[:, 0:HNT - 1], lhsT=sprev[:],
                                         rhs=p_pad[:, 0:HNT - 1], start=False,
                                         stop=False)
                        nc.tensor.matmul(w_ps[:, 0:HNT - 1], lhsT=snext[:],
                                         rhs=p_pad[:, 2:HNT + 1], start=False,
                                         stop=True)
                        nc.scalar.copy(out=w_pm[:, 0:HNT - 1],
                                       in_=w_ps[:, 0:HNT - 1])
                        tail_ts = range(0, HNT - 1)
                    else:
                        nc.tensor.matmul(w_ps[:, HNT - 1:NT - 1], lhsT=s4[:],
                                         rhs=p_pad[:, HNT:NT], start=True,
                                         stop=False)
                        nc.tensor.matmul(w_ps[:, HNT - 1:NT - 1],
                                         lhsT=sprev[:],
                                         rhs=p_pad[:, HNT - 1:NT - 1],
                                         start=False, stop=False)
                        nc.tensor.matmul(w_ps[:, HNT - 1:NT - 1],
                                         lhsT=snext[:],
                                         rhs=p_pad[:, HNT + 1:NT + 1],
                                         start=False, stop=True)
                        nc.tensor.matmul(w_ps[:, NT - 1:NT], lhsT=s4last[:],
                                         rhs=p_pad[:, NT:NT + 1], start=True,
                                         stop=False)
                        nc.tensor.matmul(w_ps[:, NT - 1:NT], lhsT=sprev[:],
                                         rhs=p_pad[:, NT - 1:NT], start=False,
                                         stop=True)
                        nc.scalar.copy(out=w_pm[:, HNT - 1:NT],
                                       in_=w_ps[:, HNT - 1:NT])
                        tail_ts = range(HNT - 1, NT)

                    for t in tail_ts:
                        xt = xh[t // HNT][:, (t % HNT) * D:(t % HNT + 1) * D]
                        for dh, pe in enumerate((pe0, pe1)):
                            nc.tensor.matmul(
                                pe[:], lhsT=w_pm[:, t:t + 1],
                                rhs=xt[:, dh * 512:(dh + 1) * 512],
                                start=(t == 0), stop=(t == NT - 1),
                            )

                # Z' = Q * (sum over partitions and halves): q_col folds the
                # 1/Q into the normalizer for free
                z_ps = psum_pool.tile([1, 1], F32, tag="pr")
                nc.tensor.matmul(z_ps[:], lhsT=zc[0][:], rhs=q_col[:],
                                 start=True, stop=False)
                nc.tensor.matmul(z_ps[:], lhsT=zc[1][:], rhs=q_col[:],
                                 start=False, stop=True)
                z2 = tiny_pool.tile([1, 1], F32, tag="z2")
                nc.scalar.copy(out=z2[:], in_=z_ps[:])

                # the reciprocal (DVE) and the PE-tail-dependent scales (ACT)
                # are deferred two batches so they never stall the in-order
                # engine queues between batches
                def emit_scales():
                    rz = tiny_pool.tile([1, 1], F32, tag="rz")
                    nc.vector.reciprocal(rz[:], z2[:])
                    nc.scalar.activation(
                        out=enc_sb[:, b * D:b * D + 512], in_=pe0[:],
                        func=AFT.Copy, scale=rz[:],
                    )
                    nc.scalar.activation(
                        out=enc_sb[:, b * D + 512:(b + 1) * D], in_=pe1[:],
                        func=AFT.Copy, scale=rz[:],
                    )
                return emit_scales

            def body_b(aT_sb=None):
                # ---- Phase B: per-batch attention ----
                # n_dve: reduce-halves handed to DVE tensor_reduce instead of
                # ACT accumulate; n_gps: muls offloaded to GpSimd.
                if logits_mode == "ttr":
                    enc_sb = small_pool.tile([1, BPC * D], F32, tag="enc_sb")
                    pending = []
                for b in range(n_batches):
                    if xhalves:
                        # half-batch tiles, DMAed in pieces: compute chases
                        # the bus.  The final half of the final batch lands
                        # in 2-tile pieces so the drain after the last byte
                        # is just a couple of logits ops + the short tail.
                        xh = []
                        for h in range(2):
                            xt_ = x_pool.tile([128, HNT * D], BF16, tag=f"xh{h}")
                            pieces = xpieces
                            if logits_mode == "ttr" and h == 1 and \
                                    b == n_batches - 1:
                                pieces = max(4, xpieces)
                            per = HNT // pieces
                            for i in range(pieces):
                                nc.sync.dma_start(
                                    out=xt_[:, i * per * D:(i + 1) * per * D],
                                    in_=xs[:, b, (h * HNT + i * per) * D:
                                           (h * HNT + (i + 1) * per) * D],
                                )
                            xh.append(xt_)
                    else:
                        xb = x_pool.tile([128, NT * D], BF16, tag="xh0")
                        nc.sync.dma_start(out=xb[:], in_=xs[:, b, 0:NT * D])
                        xh = [xb[:, 0:HNT * D], xb[:, HNT * D:NT * D]]

                    if logits_mode == "ttr":
                        if aT_sb is not None and b + 2 < n_batches:
                            replicate_a(b + 2, aT_sb, eng="act")
                        if len(pending) >= 2:
                            pending.pop(0)()
                        pending.append(ttr_batch(b, xh, enc_sb))
                        continue

                    # logits[m] = x[m, :] . a  - DVE mul per m-tile, then the
                    # free-dim reduce as two 512-halves on ACT (420ns each vs
                    # 1439ns for a 1024 accum / 1266ns for a DVE reduce).
                    logits_a = tiny_pool.tile([128, NT], F32, tag="logits_a")
                    logits_b = tiny_pool.tile([128, NT], F32, tag="logits_b")
                    if "logits" in skip:
                        nc.vector.memset(logits_a[:], 0.005)
                        nc.vector.memset(logits_b[:], 0.005)
                    elif fuse_g:
                        # G-tile fused DVE ops: one mul + one strided reduce
                        # per group amortizes the ~160-cycle DVE op overhead.
                        G = fuse_g
                        for g in range(NT // G):
                            t0 = g * G
                            h = t0 // HNT
                            xt = xh[h][:, (t0 % HNT) * D:(t0 % HNT + G) * D]
                            scratch = scr_pool.tile([128, G * D], BF16,
                                                    tag="scratch")
                            nc.vector.tensor_mul(
                                scratch[:].rearrange("p (g d) -> p g d", g=G),
                                xt.rearrange("p (g d) -> p g d", g=G),
                                a_rep[b][:, None, :].broadcast_to([128, G, D]),
                            )
                            if g < n_actg:
                                # ACT takes this group's second halves too
                                for u in range(G):
                                    nc.scalar.activation(
                                        out=scratch[:, u * D + 512:(u + 1) * D],
                                        in_=scratch[:, u * D + 512:(u + 1) * D],
                                        func=AFT.Copy,
                                        accum_out=logits_b[:, t0 + u:t0 + u + 1],
                                    )
                            else:
                                sv = scratch[:].rearrange(
                                    "p (g two h) -> p g two h", g=G, two=2)
                                nc.vector.tensor_reduce(
                                    out=logits_b[:, t0:t0 + G],
                                    in_=sv[:, :, 1, :],
                                    axis=mybir.AxisListType.X, op=ALU.add,
                                )
                            for u in range(G):
                                nc.scalar.activation(
                                    out=scratch[:, u * D:u * D + 512],
                                    in_=scratch[:, u * D:u * D + 512],
                                    func=AFT.Copy,
                                    accum_out=logits_a[:, t0 + u:t0 + u + 1],
                                )
                    else:
                        for t in range(NT):
                            xt = xh[t // HNT][:, (t % HNT) * D:(t % HNT + 1) * D]
                            scratch = scr_pool.tile([128, D], BF16, tag="scratch")
                            mul_eng = nc.gpsimd if t < n_gps else nc.vector
                            mul_eng.tensor_mul(scratch[:], xt, a_rep[b][:])
                            if t < n_dve - NT:
                                nc.vector.tensor_reduce(
                                    out=logits_a[:, t:t + 1],
                                    in_=scratch[:, 0:512],
                                    axis=mybir.AxisListType.X, op=ALU.add,
                                )
                            else:
                                if accum_dump == "psum":
                                    dmp = psum_pool.tile([128, 512], F32,
                                                         tag="pa0")
                                    outa = dmp[:]
                                elif accum_dump == "sbuf":
                                    dmp = scr_pool.tile([128, 512], BF16,
                                                        tag="dump")
                                    outa = dmp[:]
                                else:
                                    outa = scratch[:, 0:512]
                                nc.scalar.activation(
                                    out=outa, in_=scratch[:, 0:512],
                                    func=AFT.Copy, accum_out=logits_a[:, t:t + 1],
                                )
                            if t < n_dve:
                                nc.vector.tensor_reduce(
                                    out=logits_b[:, t:t + 1],
                                    in_=scratch[:, 512:1024],
                                    axis=mybir.AxisListType.X, op=ALU.add,
                                )
                            else:
                                if accum_dump == "psum":
                                    dmp = psum_pool.tile([128, 512], F32,
                                                         tag="pa1")
                                    outb = dmp[:]
                                elif accum_dump == "sbuf":
                                    dmp = scr_pool.tile([128, 512], BF16,
                                                        tag="dump")
                                    outb = dmp[:]
                                else:
                                    outb = scratch[:, 512:1024]
                                nc.scalar.activation(
                                    out=outb, in_=scratch[:, 512:1024],
                                    func=AFT.Copy,
                                    accum_out=logits_b[:, t:t + 1],
                                )
                    if logits_mode != "ttr":
                        nc.vector.tensor_add(logits_a[:], logits_a[:],
                                             logits_b[:])

                    if stop_after == "logits":
                        nc.sync.dma_start(out=enc[0, b * D:b * D + NT], in_=logits_a[0:1, :])
                        continue

                    if "softmax" in skip:
                        zsum = tiny_pool.tile([1, 1], F32, tag="zsum")
                        nc.vector.memset(zsum[:], 1.0)
                        w_pm = tiny_pool.tile([128, NT], BF16, tag="w_pm")
                        nc.vector.memset(w_pm[:], 0.01)
                        do_tail(b, xh, w_pm, zsum)
                        continue

                    # softmax without the row max: fixed shift (cancels in
                    # enc = sum(W x)/(Q Z)); exp on ACT in [128, NT] space.
                    p_pad = tiny_pool.tile([128, NT + 2], F32, tag="p_pad")
                    zcol = tiny_pool.tile([128, 1], F32, tag="zcol")
                    nc.vector.memset(p_pad[:, 0:1], 0.0)
                    nc.vector.memset(p_pad[:, NT + 1:NT + 2], 0.0)
                    nc.scalar.activation(
                        out=p_pad[:, 1:NT + 1],
                        in_=logits_a[:],
                        func=AFT.Exp,
                        bias=nshift[:],
                        scale=1.0,
                    )
                    nc.vector.tensor_reduce(
                        out=zcol[:], in_=p_pad[:, 1:NT + 1],
                        axis=mybir.AxisListType.X, op=ALU.add,
                    )

                    # Z = sum over partitions of zcol (ones-column matmul)
                    z_ps = psum_pool.tile([1, 1], F32, tag="pr")
                    nc.tensor.matmul(z_ps[:], lhsT=zcol[:], rhs=ones_col[:],
                                     start=True, stop=True)
                    zsum = tiny_pool.tile([1, 1], F32, tag="zsum")
                    nc.scalar.copy(out=zsum[:], in_=z_ps[:])

                    # W = 4-tap window of p: banded matmuls over tiles
                    # 0..14 at once (halo via shifted rhs columns of p_pad);
                    # the last tile separately with s4last (W[M-1] = 0).
                    w_ps = psum_pool.tile([128, NT], F32, tag="w_ps")
                    nc.tensor.matmul(w_ps[:, 0:NT - 1], lhsT=s4[:],
                                     rhs=p_pad[:, 1:NT], start=True, stop=False)
                    nc.tensor.matmul(w_ps[:, 0:NT - 1], lhsT=sprev[:],
                                     rhs=p_pad[:, 0:NT - 1], start=False,
                                     stop=False)
                    nc.tensor.matmul(w_ps[:, 0:NT - 1], lhsT=snext[:],
                                     rhs=p_pad[:, 2:NT + 1], start=False,
                                     stop=True)
                    nc.tensor.matmul(w_ps[:, NT - 1:NT], lhsT=s4last[:],
                                     rhs=p_pad[:, NT:NT + 1], start=True,
                                     stop=False)
                    nc.tensor.matmul(w_ps[:, NT - 1:NT], lhsT=sprev[:],
                                     rhs=p_pad[:, NT - 1:NT], start=False,
                                     stop=True)
                    w_pm = tiny_pool.tile([128, NT], BF16, tag="w_pm")
                    nc.scalar.copy(out=w_pm[:], in_=w_ps[:])

                    do_tail(b, xh, w_pm, zsum)

                if logits_mode == "ttr":
                    for f in pending:
                        f()
                    # NOTE: issuing this from the ACT DGE queue instead was
                    # measured 23us/rep SLOWER on HW (143 vs 120 at
                    # unroll=8) — keep it on the SP sync queue
                    nc.sync.dma_start(out=enc[:], in_=enc_sb[0:1, :])

            def do_tail(b, xh, w_pm, zsum):
                # enc_un[d] = sum_m W[m] x[m, d]   (PE, W cols as weights)
                pe0 = psum_e_pool.tile([1, 512], F32, tag="pe0")
                pe1 = psum_e_pool.tile([1, 512], F32, tag="pe1")
                for t in range(NT):
                    xt = xh[t // HNT][:, (t % HNT) * D:(t % HNT + 1) * D]
                    for dh, pe in enumerate((pe0, pe1)):
                        nc.tensor.matmul(
                            pe[:],
                            lhsT=w_pm[:, t:t + 1],
                            rhs=xt[:, dh * 512:(dh + 1) * 512],
                            start=(t == 0),
                            stop=(t == NT - 1),
                        )

                enc_sb = small_pool.tile([1, BPC * D], F32, tag="enc_sb")
                if stop_after == "mm":
                    nc.scalar.copy(out=enc_sb[:, b * D:b * D + 512], in_=pe0[:])
                    nc.scalar.copy(out=enc_sb[:, b * D + 512:(b + 1) * D],
                                   in_=pe1[:])
                    if b == n_batches - 1:
                        nc.sync.dma_start(out=enc[:], in_=enc_sb[0:1, :])
                    return

                # enc[b] = enc_un / (Q * Z)
                z2 = small_pool.tile([1, 1], F32, tag="z2")
                nc.scalar.mul(out=z2[:], in_=zsum[:], mul=float(Q))
                rz = small_pool.tile([1, 1], F32, tag="rz")
                nc.vector.reciprocal(rz[:], z2[:])
                nc.scalar.activation(
                    out=enc_sb[:, b * D:b * D + 512], in_=pe0[:], func=AFT.Copy,
                    scale=rz[:],
                )
                nc.scalar.activation(
                    out=enc_sb[:, b * D + 512:(b + 1) * D], in_=pe1[:],
                    func=AFT.Copy, scale=rz[:],
                )
                if b == n_batches - 1:
                    nc.sync.dma_start(out=enc[:], in_=enc_sb[0:1, :])

            if reps == 1:
                body()
            elif unroll == 0:
                # flat python unroll: no hardware loop at all (timeline-sim
                # and collective-bearing builds can't use For_i)
                for _ in range(reps):
                    body()
            elif unroll > 1:
                # unrolled loop body: consecutive reps rotate through the
                # tile pools, letting the next rep's P/x DMAs overlap this
                # rep's batch compute (the plain loop edge serializes; a
                # non-loop epilogue body also kills the pipelining, so the
                # rep count rounds UP to a multiple of unroll - reps are
                # idempotent, an extra one only costs time).
                with tc.For_i(0, (reps + unroll - 1) // unroll, 1):
                    for _ in range(unroll):
                        body()
            else:
                with tc.For_i(0, reps, 1):
                    body()

    return nc


def _shard_inputs(embeds_x, embeds_y, P, pshard=False):
    """Build the 8 per-core input maps (host-side layout + bf16 cast)."""
    bf = ml_dtypes.bfloat16
    x = np.asarray(embeds_x, dtype=np.float32)
    y = np.asarray(embeds_y, dtype=np.float32)[:, :, 0]          # [B, CD]
    if pshard:
        ptr_full = P.T.reshape(KT, 128, D).astype(bf)            # [KT, 128, D]
        yk = y.reshape(B, KT, 128).astype(bf)                    # [B, KT, 128]
        in_maps = []
        for c in range(NCORES):
            kt0 = c * KTC
            pt_c = np.zeros((128, KTC * D + PAD), dtype=bf)
            pt_c[:, :KTC * D] = ptr_full[kt0:kt0 + KTC].transpose(
                1, 0, 2).reshape(128, KTC * D)
            ys_c = np.ascontiguousarray(
                yk[:, kt0:kt0 + KTC, :].transpose(2, 1, 0))      # [128, KTC, B]
            sel_c = np.zeros((B, BPC * 128), dtype=bf)
            for b in range(BPC):
                sel_c[c * BPC + b, b * 128:(b + 1) * 128] = 1.0
            sl = slice(c * BPC, (c + 1) * BPC)
            xs_c = np.zeros((128, BPC, NT * D + PAD), dtype=bf)
            xs_c[:, :, :NT * D] = x[sl].reshape(BPC, NT, 128, D).transpose(
                2, 0, 1, 3).reshape(128, BPC, NT * D).astype(bf)
            in_maps.append({"xs": xs_c, "pt": pt_c, "ys": ys_c, "sel": sel_c})
        return in_maps
    # pt[p, k*D + d] = P[d, k*128 + p]
    ptr = np.zeros((128, KT * D + PAD), dtype=bf)
    ptr[:, :KT * D] = P.T.reshape(KT, 128, D).transpose(1, 0, 2).reshape(
        128, KT * D).astype(bf)
    in_maps = []
    for c in range(NCORES):
        sl = slice(c * BPC, (c + 1) * BPC)
        # xs[p, b, t*D + d] = x[b, t*128 + p, d]
        xs_c = np.zeros((128, BPC, NT * D + PAD), dtype=bf)
        xs_c[:, :, :NT * D] = x[sl].reshape(BPC, NT, 128, D).transpose(
            2, 0, 1, 3).reshape(128, BPC, NT * D).astype(bf)
        ys_c = np.ascontiguousarray(
            y[sl].reshape(BPC, KT, 128).transpose(2, 1, 0)
        ).astype(bf)  # [128, KT, BPC]
        in_maps.append({"xs": xs_c, "pt": ptr, "ys": ys_c})
    return in_maps


# canonical build configuration — kernel() and test.py both use this
BUILD_KW = dict(xhalves=True, logits_mode="ttr", n_pool=5, xbufs=3,
                ptbufs=4, unroll=8)


def kernel(embeds_x, embeds_y, P, M):
    assert int(M) == 2048
    nc = build_nc(reps=1, **BUILD_KW)
    split_sync_waits(nc)  # HW-compile only; CoreSim rejects injected NoOps
    in_maps = _shard_inputs(embeds_x, embeds_y, P)
    res = run_bass_kernel_spmd(nc, in_maps, list(range(NCORES)))
    out = np.concatenate(
        [res.results[c]["enc"].reshape(BPC, D) for c in range(NCORES)], axis=0)
    return out.astype(np.float32)



# revision 44
# speedup vs baseline: 1.4625x; 1.4625x over previous
"""Trainium2 Bass kernel for the attention-based encoder.

Computation (per batch b):
    a      = P @ y[b]                                  # [D]
    logits = x[b] @ a                                  # [M]
    p_un   = exp(logits - 16)                          # fixed shift (cancels)
    Z      = sum(p_un)
    W[t]   = p_un[t-1] + p_un[t] + p_un[t+1] + p_un[t+2]  (zero-padded), W[M-1] = 0
    enc[b] = (W @ x[b]) / (Q * Z)                      # [D]

Design (each point validated by HW ablation / TimelineSim analysis):
  * all HBM traffic in bf16 (x 16 MiB + P 10 MiB per core; rel err 3.3e-3
    vs the 2e-2 gate) - halves the DMA floor and runs every matmul at
    bf16 rate instead of multi-pass fp32.
  * host pre-arranges x and P^T so every DMA is one large transfer with
    contiguous 16-80 KiB per partition, and pads the per-partition row
    by 2176 B: the power-of-2 HBM stride otherwise aliases banks
    (187 -> 315 GB/s measured).
  * logits via ONE fused op per m-tile: DVE scalar_tensor_tensor
    (out=(x*1)*a_rep, accum_out=full-1024 dot), ALL 16 tiles on DVE.
    (tensor_tensor_reduce and Pool-side STT both fail walrus codegen
    "ISA wrong length" — only DVE STT works.  Same-process HW sweep:
    n_pool=0/3/5 -> 97.6/113.7/120.9 us per rep — every tile moved to
    the Pool-mul + ACT-accumulate path COSTS ~4.7 us/rep despite the
    cost model predicting it balances; ablating the whole logits stage
    hits the 86.6 us DMA floor, so logits is the only stage above it.)
  * per-batch split-half pipeline: tiles 0-7 logits -> exp(+Z accum) ->
    W cols 0..6 -> PE tail t0..6 overlap tiles 8-15 logits; W = 4-tap
    window of p via banded matmuls with +-2 halo columns.
  * x halves DMAed in 2 pieces (last batch's h1 in 4) so compute chases
    the bus; last P chunk's DMA split 7+1 k-tiles so phase A trails the
    29-us P transfer by <1 us.
  * a replicated across partitions with a selector matmul; copies ride
    the idle ACT engine, scheduled b0/b1 in phase A and b+2 inside
    batch b (keeps the single 'pr' PSUM bank ring short without the PE
    tail ever gating a batch start).
  * the normalizer chain is deferred: Q folded into the Z ones-matmul
    weights, reciprocal (DVE) + PSUM->enc scales (ACT, dep on PE tail
    stop) of batch b emitted at batch b+2 so the in-order DVE/ACT
    queues never stall between batches.
  * reps>1 builds unroll the For_i body by 8 (HW: 126.9/122.9/120.4 us
    per rep at unroll 2/4/8).  Flat (no-loop) unrolling collapses to
    1.7 ms/rep on HW (instruction-memory thrash) — loops are mandatory.
    Emitting the output DMA from the ACT DGE queue instead of SP was
    23 us/rep slower — keep every DMA on nc.sync.
  * pshard=True phase A (P split over cores + AllReduce) works at
    reps=1 and is equally accurate (3.298e-3), but collectives fail at
    runtime inside a For_i reps-loop, so it stays off.

Sharding: data-parallel over batch, 4 batches per core on 8 cores.
"""

import numpy as np
import ml_dtypes

import concourse.bass as bass
import concourse.mybir as mybir
from concourse.tile import TileContext
from concourse.bass_utils import run_bass_kernel_spmd

# ---------------------------------------------------------------------------
# This container's walrus supports only ONE sync wait per instruction ("Too
# many sync wait commands" at codegen otherwise), while Tile freely attaches
# several.  Post-pass: hoist excess waits onto injected same-engine NoOps
# placed immediately before the over-subscribed instruction.
# ---------------------------------------------------------------------------

_MAX_WAITS = 1


def split_sync_waits(nc: bass.Bass) -> None:
    uid = 0
    for fn in nc.m.functions:
        for blk in fn.blocks:
            new_insts = []
            for inst in blk.instructions:
                si = inst.sync_info
                waits = list(si.on_wait) if si and si.on_wait else []
                if len(waits) > _MAX_WAITS:
                    for w in waits[:-_MAX_WAITS]:
                        uid += 1
                        ev = mybir.InstEventSemaphore(
                            name=f"{inst.name}_hw{uid}",
                            opcode="EventSemaphore",
                            ins=[],
                            outs=[],
                            sync_info=mybir.SyncInfo(on_wait=[w], on_update=[]),
                        )
                        ev.engine = inst.engine
                        new_insts.append(ev)
                    si.on_wait = waits[-_MAX_WAITS:]
                new_insts.append(inst)
            blk.instructions[:] = new_insts

# ---------------------------------------------------------------------------

B, M, D, CD = 32, 2048, 1024, 5120
Q = 2
NCORES = 8
BPC = B // NCORES          # batches per core
NT = M // 128              # m-tiles per batch
KT = CD // 128             # k-tiles of the P contraction
KCH = 8                    # k-tiles per P DMA chunk
NCH = KT // KCH
KTC = KT // NCORES  # k-tiles per core when P is sharded
PAD = 1088                 # bf16 elems of per-partition-row padding: breaks
                           # the power-of-2 HBM stride aliasing (187 -> 315 GB/s)
HNT = NT // 2              # m-tiles per xb half-load
F32 = mybir.dt.float32
BF16 = mybir.dt.bfloat16
ALU = mybir.AluOpType
AFT = mybir.ActivationFunctionType


def build_nc(reps: int = 1, n_batches: int = BPC, do_phase_b: bool = True,
             stop_after: str = "full", skip: tuple = (),
             logits_mode: str = "split", xhalves: bool = True,
             n_dve: int = 16, n_gps: int = 0,
             accum_dump: str = "inplace", fuse_g: int = 4,
             n_actg: int = 0, pshard: bool = False,
             unroll: int = 2, n_pool: int = 0, xpieces: int = 2,
             xbufs: int = 3, ptbufs: int = 3, scrbufs: int = 3) -> bass.Bass:
    nc = bass.Bass(num_devices=NCORES) if pshard else bass.Bass()
    xs = nc.declare_dram_parameter("xs", [128, BPC, NT * D + PAD], BF16, isOutput=False)
    if pshard:
        # P sharded over cores: each core holds KTC k-tiles of P^T, the y
        # slices matching those k-tiles for ALL B batches, and a selector
        # that picks this core's batch rows out of the allreduced aT.
        pt = nc.declare_dram_parameter("pt", [128, KTC * D + PAD], BF16,
                                       isOutput=False)
        ys = nc.declare_dram_parameter("ys", [128, KTC, B], BF16,
                                       isOutput=False)
        sel = nc.declare_dram_parameter("sel", [B, BPC * 128], BF16,
                                        isOutput=False)
        cc_in = nc.dram_tensor("cc_in", [B, D], F32, addr_space="Local")
        cc_out = nc.dram_tensor("cc_out", [B, D], F32, addr_space="Local")
    else:
        pt = nc.declare_dram_parameter("pt", [128, KT * D + PAD], BF16,
                                       isOutput=False)
        ys = nc.declare_dram_parameter("ys", [128, KT, BPC], BF16,
                                       isOutput=False)
    enc = nc.declare_dram_parameter("enc", [1, BPC * D], F32, isOutput=True)

    with TileContext(nc) as tc:
        with (
            tc.tile_pool(name="const", bufs=1) as const_pool,
            tc.tile_pool(name="ysp", bufs=1) as ys_pool,
            tc.tile_pool(name="ptp", bufs=ptbufs) as pt_pool,
            tc.tile_pool(name="xp", bufs=xbufs) as x_pool,
            tc.tile_pool(name="arep", bufs=1) as arep_pool,
            tc.tile_pool(name="small", bufs=1) as small_pool,
            tc.tile_pool(name="tiny", bufs=2) as tiny_pool,
            tc.tile_pool(name="scr", bufs=scrbufs) as scr_pool,
            tc.tile_pool(name="ps", bufs=1, space="PSUM") as psum_pool,
            tc.tile_pool(name="pse", bufs=2, space="PSUM") as psum_e_pool,
        ):
            ones_col = const_pool.tile([128, 1], F32)
            nc.vector.memset(ones_col[:], 1.0)
            # Q-scaled ones: folds the x_bar 1/Q into Z at no cost
            q_col = const_pool.tile([128, 1], F32)
            nc.vector.memset(q_col[:], float(Q))
            nshift = const_pool.tile([128, 1], F32)
            nc.vector.memset(nshift[:], -16.0)
            if pshard:
                ys_sb = const_pool.tile([128, KTC, B], BF16)
                nc.sync.dma_start(out=ys_sb[:], in_=ys[:])
                sel_sb = const_pool.tile([B, BPC * 128], BF16)
                nc.sync.dma_start(out=sel_sb[:], in_=sel[:])
            else:
                ys_sb = const_pool.tile([128, KT, BPC], BF16)
                nc.sync.dma_start(out=ys_sb[:], in_=ys[:])

            # banded matrices for the 4-tap sliding-window sum W = S4 @ p:
            # s4[c, f] = 1 iff f - c in {-2, -1, 0, 1}; corner matrices
            # carry the +-2-element inter-tile halo.
            s4 = const_pool.tile([128, 128], F32)
            nc.gpsimd.memset(s4[:], 0.0)
            for base in (1, 0, -1, -2):
                nc.gpsimd.affine_select(
                    out=s4[:], in_=s4[:], compare_op=ALU.not_equal, fill=1.0,
                    base=base, pattern=[[-1, 128]], channel_multiplier=1,
                )
            sprev = const_pool.tile([128, 128], F32)
            nc.gpsimd.memset(sprev[:], 0.0)
            nc.gpsimd.affine_select(
                out=sprev[:], in_=sprev[:], compare_op=ALU.not_equal, fill=1.0,
                base=-127, pattern=[[-1, 128]], channel_multiplier=1,
            )
            snext = const_pool.tile([128, 128], F32)
            nc.gpsimd.memset(snext[:], 0.0)
            for base in (126, 127):
                nc.gpsimd.affine_select(
                    out=snext[:], in_=snext[:], compare_op=ALU.not_equal, fill=1.0,
                    base=base, pattern=[[-1, 128]], channel_multiplier=1,
                )

            # last-tile variant of s4 with column M-1 zeroed (W[M-1] = 0)
            s4last = const_pool.tile([128, 128], F32)
            nc.gpsimd.memset(s4last[:], 0.0)
            for base in (1, 0, -1, -2):
                nc.gpsimd.affine_select(
                    out=s4last[:], in_=s4last[:], compare_op=ALU.not_equal,
                    fill=1.0, base=base, pattern=[[-1, 128]],
                    channel_multiplier=1,
                )
            nc.gpsimd.affine_select(
                out=s4last[:], in_=s4last[:], compare_op=ALU.not_equal,
                fill=0.0, base=-127, pattern=[[1, 128]], channel_multiplier=0,
            )

            # selector rows: selb[b][k, m] = 1 iff k == b (4 partitions);
            # a_rep[b] = selb[b].T @ aT broadcasts batch b's row of aT to
            # all 128 partitions without an SBUF round-trip.
            selb = []
            for b in range(BPC):
                sb = const_pool.tile([BPC, 128], BF16, name=f"selb{b}")
                nc.gpsimd.memset(sb[:], 0.0)
                nc.gpsimd.affine_select(
                    out=sb[:], in_=sb[:], compare_op=ALU.not_equal, fill=1.0,
                    base=-b, pattern=[[0, 128]], channel_multiplier=1,
                )
                selb.append(sb)

            a_rep = [
                arep_pool.tile([128, D], BF16, tag=f"a_rep{b}", name=f"a_rep{b}")
                for b in range(BPC)
            ]

            def body(_=None):
                if "phase_a" in skip:
                    [nc.vector.memset(ar[:], 0.001) for ar in a_rep]
                    return body_b()

                if pshard:
                    return phase_a_sharded()

                # ---- Phase A: aT[b, d] = sum_k y[b, k] * PT[k, d] ----
                pa0 = psum_pool.tile([BPC, 512], F32, tag="pa0")
                pa1 = psum_pool.tile([BPC, 512], F32, tag="pa1")
                for c in range(NCH):
                    ptc = pt_pool.tile([128, KCH * D], BF16, tag="ptc")
                    if c == NCH - 1:
                        # split the last chunk's DMA so its final k-tile (and
                        # the phase-A tail) lags the P transfer by ~0.8us
                        # instead of a full 2MiB chunk
                        nc.sync.dma_start(
                            out=ptc[:, 0:(KCH - 1) * D],
                            in_=pt[:, c * KCH * D:(c * KCH + KCH - 1) * D],
                        )
                        nc.sync.dma_start(
                            out=ptc[:, (KCH - 1) * D:KCH * D],
                            in_=pt[:, (c * KCH + KCH - 1) * D:(c + 1) * KCH * D],
                        )
                    else:
                        nc.sync.dma_start(
                            out=ptc[:], in_=pt[:, c * KCH * D:(c + 1) * KCH * D]
                        )
                    for u in range(KCH):
                        t = c * KCH + u
                        nc.tensor.matmul(
                            pa0[:], lhsT=ys_sb[:, t, :],
                            rhs=ptc[:, u * D:u * D + 512],
                            start=(t == 0), stop=(t == KT - 1),
                        )
                        nc.tensor.matmul(
                            pa1[:], lhsT=ys_sb[:, t, :],
                            rhs=ptc[:, u * D + 512:(u + 1) * D],
                            start=(t == 0), stop=(t == KT - 1),
                        )
                aT_sb = small_pool.tile([BPC, D], BF16, tag="aT")
                nc.scalar.copy(out=aT_sb[:, 0:512], in_=pa0[:])
                nc.scalar.copy(out=aT_sb[:, 512:1024], in_=pa1[:])

                if logits_mode == "ttr":
                    # replicate a for the first two batches now (copies on
                    # the mostly-idle ACT engine); b+2's replication is
                    # emitted inside batch b so the PSUM 'pr' ring stays
                    # short while batch starts never wait on the PE tail
                    for b in range(min(2, n_batches if do_phase_b else BPC)):
                        replicate_a(b, aT_sb, eng="act")
                else:
                    for b in range(BPC):
                        replicate_a(b, aT_sb, eng="dve")

                if not do_phase_b:
                    for b in range(BPC):
                        if logits_mode == "ttr" and b >= 2:
                            replicate_a(b, aT_sb, eng="act")
                        nc.gpsimd.dma_start(
                            out=enc[0, b * D:b * D + 512],
                            in_=a_rep[b][0:1, 0:512],
                        )
                    return
                return body_b(aT_sb)

            def replicate_a(b, aT_sb, eng):
                for dh in range(2):
                    pr = psum_pool.tile([128, 512], F32, tag="pr")
                    nc.tensor.matmul(
                        pr[:], lhsT=selb[b][:],
                        rhs=aT_sb[:, dh * 512:(dh + 1) * 512],
                        start=True, stop=True,
                    )
                    if eng == "act":
                        nc.scalar.copy(
                            out=a_rep[b][:, dh * 512:(dh + 1) * 512],
                            in_=pr[:],
                        )
                    else:
                        nc.vector.tensor_copy(
                            a_rep[b][:, dh * 512:(dh + 1) * 512], pr[:]
                        )

            def phase_a_sharded():
                # partial aT for ALL batches from this core's P k-tiles,
                # then an 8-core AllReduce combines the partials.
                pa0 = psum_pool.tile([B, 512], F32, tag="pa0")
                pa1 = psum_pool.tile([B, 512], F32, tag="pa1")
                ptc = pt_pool.tile([128, KTC * D], BF16, tag="ptc")
                nc.sync.dma_start(out=ptc[:], in_=pt[:, 0:KTC * D])
                for kt in range(KTC):
                    nc.tensor.matmul(
                        pa0[:], lhsT=ys_sb[:, kt, :],
                        rhs=ptc[:, kt * D:kt * D + 512],
                        start=(kt == 0), stop=(kt == KTC - 1),
                    )
                    nc.tensor.matmul(
                        pa1[:], lhsT=ys_sb[:, kt, :],
                        rhs=ptc[:, kt * D + 512:(kt + 1) * D],
                        start=(kt == 0), stop=(kt == KTC - 1),
                    )
                aT_part = small_pool.tile([B, D], F32, tag="aT_part")
                nc.vector.tensor_copy(aT_part[:, 0:512], pa0[:])
                nc.vector.tensor_copy(aT_part[:, 512:1024], pa1[:])
                nc.sync.dma_start(out=cc_in[:], in_=aT_part[:])
                nc.gpsimd.collective_compute(
                    "AllReduce", ALU.add,
                    replica_groups=[list(range(NCORES))],
                    ins=[cc_in[:]], outs=[cc_out[:]],
                )
                aT_f32 = small_pool.tile([B, D], F32, tag="aT_f32")
                nc.sync.dma_start(out=aT_f32[:], in_=cc_out[:])
                aT_sb = small_pool.tile([B, D], BF16, tag="aT")
                nc.vector.tensor_copy(aT_sb[:], aT_f32[:])

                for b in range(BPC):
                    for dh in range(2):
                        pr = psum_pool.tile([128, 512], F32, tag="pr")
                        nc.tensor.matmul(
                            pr[:], lhsT=sel_sb[:, b * 128:(b + 1) * 128],
                            rhs=aT_sb[:, dh * 512:(dh + 1) * 512],
                            start=True, stop=True,
                        )
                        nc.vector.tensor_copy(
                            a_rep[b][:, dh * 512:(dh + 1) * 512], pr[:]
                        )
                if not do_phase_b:
                    for b in range(BPC):
                        nc.gpsimd.dma_start(
                            out=enc[0, b * D:b * D + 512],
                            in_=a_rep[b][0:1, 0:512],
                        )
                    return
                return body_b()

            def ttr_batch(b, xh, enc_sb):
                # fused logits: one tensor_tensor_reduce (DVE) or
                # scalar_tensor_tensor (Pool) per m-tile — accum_out takes
                # the full 1024-wide dot against a_rep.  The batch is
                # processed in two m-halves so exp/W/tail of half 0 overlap
                # the second half's logits ops (cuts the per-batch drain).
                logits = tiny_pool.tile([128, NT], F32, tag="logits_a")
                p_pad = tiny_pool.tile([128, NT + 2], F32, tag="p_pad")
                w_pm = tiny_pool.tile([128, NT], BF16, tag="w_pm")
                zc = [tiny_pool.tile([128, 1], F32, tag=f"zc{h}",
                                     name=f"zc{h}") for h in range(2)]
                if b < 2:
                    # pad columns are zeroed once per ring buffer (bufs=2);
                    # later batches reuse the already-zeroed slots
                    nc.vector.memset(p_pad[:, 0:1], 0.0)
                    nc.vector.memset(p_pad[:, NT + 1:NT + 2], 0.0)
                if "ttr_tail" not in skip:
                    w_ps = psum_pool.tile([128, NT], F32, tag="w_ps")
                    pe0 = psum_e_pool.tile([1, 512], F32, tag="pe0")
                    pe1 = psum_e_pool.tile([1, 512], F32, tag="pe1")

                for h in range(2):
                    t0, t1 = h * HNT, h * HNT + HNT
                    npool_h = (n_pool + (1 - h)) // 2
                    if "ttr_logits" in skip:
                        # ablation: constant logits, x left unconsumed by
                        # this stage (DMA traffic unchanged)
                        nc.vector.memset(logits[:, t0:t1], 0.005)
                    else:
                     for t in range(t0, t1):
                        xt = xh[h][:, (t - t0) * D:(t - t0 + 1) * D]
                        if t - t0 < npool_h:
                            # Pool mul + ACT accumulate (Pool cannot encode
                            # the fused STT-with-accum; TTR is unsupported by
                            # this walrus build entirely)
                            scr = scr_pool.tile([128, D], BF16, tag="scr_p")
                            nc.gpsimd.tensor_mul(scr[:], xt, a_rep[b][:])
                            nc.scalar.activation(
                                out=scr[:], in_=scr[:], func=AFT.Copy,
                                accum_out=logits[:, t:t + 1],
                            )
                        else:
                            # fused mul+sum in one DVE op:
                            # out=(x·1)·a, accum_out=Σ_free out
                            scr = scr_pool.tile([128, D], BF16, tag="scr_v")
                            nc.vector.scalar_tensor_tensor(
                                out=scr[:], in0=xt, scalar=1.0,
                                in1=a_rep[b][:], op0=ALU.mult, op1=ALU.mult,
                                accum_out=logits[:, t:t + 1],
                            )

                    # softmax numerator for this half (fixed shift cancels);
                    # Z partial lands in zc[h] via the ACT accumulator
                    nc.scalar.activation(
                        out=p_pad[:, 1 + t0:1 + t1], in_=logits[:, t0:t1],
                        func=AFT.Exp, bias=nshift[:], scale=1.0,
                        accum_out=zc[h][:],
                    )

                    # W columns + PE tail for this half (halo: W col c needs
                    # p_pad cols c..c+2, all present once this half's exp is
                    # done)
                    if "ttr_tail" in skip:
                        # ablation: no W / tail matmuls (scales then read
                        # stale PSUM — timing only, output is garbage)
                        continue
                    if h == 0:
                        nc.tensor.matmul(w_ps[:, 0:HNT - 1], lhsT=s4[:],
                                         rhs=p_pad[:, 1:HNT], start=True,
                                         stop=False)
                        nc.tensor.matmul(w_ps[:, 0:HNT - 1], lhsT=sprev[:],
                                         rhs=p_pad[:, 0:HNT - 1], start=False,
                                         stop=False)
                        nc.tensor.matmul(w_ps[:, 0:HNT - 1], lhsT=snext[:],
                                         rhs=p_pad[:, 2:HNT + 1], start=False,
                                         stop=True)
                        nc.scalar.copy(out=w_pm[:, 0:HNT - 1],
                                       in_=w_ps[:, 0:HNT - 1])
                        tail_ts = range(0, HNT - 1)
                    else:
                        nc.tensor.matmul(w_ps[:, HNT - 1:NT - 1], lhsT=s4[:],
                                         rhs=p_pad[:, HNT:NT], start=True,
                                         stop=False)
                        nc.tensor.matmul(w_ps[:, HNT - 1:NT - 1],
                                         lhsT=sprev[:],
                                         rhs=p_pad[:, HNT - 1:NT - 1],
                                         start=False, stop=False)
                        nc.tensor.matmul(w_ps[:, HNT - 1:NT - 1],
                                         lhsT=snext[:],
                                         rhs=p_pad[:, HNT + 1:NT + 1],
                                         start=False, stop=True)
                        nc.tensor.matmul(w_ps[:, NT - 1:NT], lhsT=s4last[:],
                                         rhs=p_pad[:, NT:NT + 1], start=True,
                                         stop=False)
                        nc.tensor.matmul(w_ps[:, NT - 1:NT], lhsT=sprev[:],
                                         rhs=p_pad[:, NT - 1:NT], start=False,
                                         stop=True)
                        nc.scalar.copy(out=w_pm[:, HNT - 1:NT],
                                       in_=w_ps[:, HNT - 1:NT])
                        tail_ts = range(HNT - 1, NT)

                    for t in tail_ts:
                        xt = xh[t // HNT][:, (t % HNT) * D:(t % HNT + 1) * D]
                        for dh, pe in enumerate((pe0, pe1)):
                            nc.tensor.matmul(
                                pe[:], lhsT=w_pm[:, t:t + 1],
                                rhs=xt[:, dh * 512:(dh + 1) * 512],
                                start=(t == 0), stop=(t == NT - 1),
                            )

                # Z' = Q * (sum over partitions and halves): q_col folds the
                # 1/Q into the normalizer for free
                z_ps = psum_pool.tile([1, 1], F32, tag="pr")
                nc.tensor.matmul(z_ps[:], lhsT=zc[0][:], rhs=q_col[:],
                                 start=True, stop=False)
                nc.tensor.matmul(z_ps[:], lhsT=zc[1][:], rhs=q_col[:],
                                 start=False, stop=True)
                z2 = tiny_pool.tile([1, 1], F32, tag="z2")
                nc.scalar.copy(out=z2[:], in_=z_ps[:])

                # the reciprocal (DVE) and the PE-tail-dependent scales (ACT)
                # are deferred two batches so they never stall the in-order
                # engine queues between batches
                def emit_scales():
                    rz = tiny_pool.tile([1, 1], F32, tag="rz")
                    nc.vector.reciprocal(rz[:], z2[:])
                    if "ttr_tail" in skip:
                        nc.gpsimd.memset(enc_sb[:, b * D:(b + 1) * D], 0.1)
                        return
                    nc.scalar.activation(
                        out=enc_sb[:, b * D:b * D + 512], in_=pe0[:],
                        func=AFT.Copy, scale=rz[:],
                    )
                    nc.scalar.activation(
                        out=enc_sb[:, b * D + 512:(b + 1) * D], in_=pe1[:],
                        func=AFT.Copy, scale=rz[:],
                    )
                return emit_scales

            def body_b(aT_sb=None):
                # ---- Phase B: per-batch attention ----
                # n_dve: reduce-halves handed to DVE tensor_reduce instead of
                # ACT accumulate; n_gps: muls offloaded to GpSimd.
                if logits_mode == "ttr":
                    enc_sb = small_pool.tile([1, BPC * D], F32, tag="enc_sb")
                    pending = []
                for b in range(n_batches):
                    if xhalves:
                        # half-batch tiles, DMAed in pieces: compute chases
                        # the bus.  The final half of the final batch lands
                        # in 2-tile pieces so the drain after the last byte
                        # is just a couple of logits ops + the short tail.
                        xh = []
                        for h in range(2):
                            xt_ = x_pool.tile([128, HNT * D], BF16, tag=f"xh{h}")
                            pieces = xpieces
                            if logits_mode == "ttr" and h == 1 and \
                                    b == n_batches - 1:
                                pieces = max(4, xpieces)
                            per = HNT // pieces
                            for i in range(pieces):
                                nc.sync.dma_start(
                                    out=xt_[:, i * per * D:(i + 1) * per * D],
                                    in_=xs[:, b, (h * HNT + i * per) * D:
                                           (h * HNT + (i + 1) * per) * D],
                                )
                            xh.append(xt_)
                    else:
                        xb = x_pool.tile([128, NT * D], BF16, tag="xh0")
                        nc.sync.dma_start(out=xb[:], in_=xs[:, b, 0:NT * D])
                        xh = [xb[:, 0:HNT * D], xb[:, HNT * D:NT * D]]

                    if logits_mode == "ttr":
                        if aT_sb is not None and b + 2 < n_batches:
                            replicate_a(b + 2, aT_sb, eng="act")
                        if len(pending) >= 2:
                            pending.pop(0)()
                        pending.append(ttr_batch(b, xh, enc_sb))
                        continue

                    # logits[m] = x[m, :] . a  - DVE mul per m-tile, then the
                    # free-dim reduce as two 512-halves on ACT (420ns each vs
                    # 1439ns for a 1024 accum / 1266ns for a DVE reduce).
                    logits_a = tiny_pool.tile([128, NT], F32, tag="logits_a")
                    logits_b = tiny_pool.tile([128, NT], F32, tag="logits_b")
                    if "logits" in skip:
                        nc.vector.memset(logits_a[:], 0.005)
                        nc.vector.memset(logits_b[:], 0.005)
                    elif fuse_g:
                        # G-tile fused DVE ops: one mul + one strided reduce
                        # per group amortizes the ~160-cycle DVE op overhead.
                        G = fuse_g
                        for g in range(NT // G):
                            t0 = g * G
                            h = t0 // HNT
                            xt = xh[h][:, (t0 % HNT) * D:(t0 % HNT + G) * D]
                            scratch = scr_pool.tile([128, G * D], BF16,
                                                    tag="scratch")
                            nc.vector.tensor_mul(
                                scratch[:].rearrange("p (g d) -> p g d", g=G),
                                xt.rearrange("p (g d) -> p g d", g=G),
                                a_rep[b][:, None, :].broadcast_to([128, G, D]),
                            )
                            if g < n_actg:
                                # ACT takes this group's second halves too
                                for u in range(G):
                                    nc.scalar.activation(
                                        out=scratch[:, u * D + 512:(u + 1) * D],
                                        in_=scratch[:, u * D + 512:(u + 1) * D],
                                        func=AFT.Copy,
                                        accum_out=logits_b[:, t0 + u:t0 + u + 1],
                                    )
                            else:
                                sv = scratch[:].rearrange(
                                    "p (g two h) -> p g two h", g=G, two=2)
                                nc.vector.tensor_reduce(
                                    out=logits_b[:, t0:t0 + G],
                                    in_=sv[:, :, 1, :],
                                    axis=mybir.AxisListType.X, op=ALU.add,
                                )
                            for u in range(G):
                                nc.scalar.activation(
                                    out=scratch[:, u * D:u * D + 512],
                                    in_=scratch[:, u * D:u * D + 512],
                                    func=AFT.Copy,
                                    accum_out=logits_a[:, t0 + u:t0 + u + 1],
                                )
                    else:
                        for t in range(NT):
                            xt = xh[t // HNT][:, (t % HNT) * D:(t % HNT + 1) * D]
                            scratch = scr_pool.tile([128, D], BF16, tag="scratch")
                            mul_eng = nc.gpsimd if t < n_gps else nc.vector
                            mul_eng.tensor_mul(scratch[:], xt, a_rep[b][:])
                            if t < n_dve - NT:
                                nc.vector.tensor_reduce(
                                    out=logits_a[:, t:t + 1],
                                    in_=scratch[:, 0:512],
                                    axis=mybir.AxisListType.X, op=ALU.add,
                                )
                            else:
                                if accum_dump == "psum":
                                    dmp = psum_pool.tile([128, 512], F32,
                                                         tag="pa0")
                                    outa = dmp[:]
                                elif accum_dump == "sbuf":
                                    dmp = scr_pool.tile([128, 512], BF16,
                                                        tag="dump")
                                    outa = dmp[:]
                                else:
                                    outa = scratch[:, 0:512]
                                nc.scalar.activation(
                                    out=outa, in_=scratch[:, 0:512],
                                    func=AFT.Copy, accum_out=logits_a[:, t:t + 1],
                                )
                            if t < n_dve:
                                nc.vector.tensor_reduce(
                                    out=logits_b[:, t:t + 1],
                                    in_=scratch[:, 512:1024],
                                    axis=mybir.AxisListType.X, op=ALU.add,
                                )
                            else:
                                if accum_dump == "psum":
                                    dmp = psum_pool.tile([128, 512], F32,
                                                         tag="pa1")
                                    outb = dmp[:]
                                elif accum_dump == "sbuf":
                                    dmp = scr_pool.tile([128, 512], BF16,
                                                        tag="dump")
                                    outb = dmp[:]
                                else:
                                    outb = scratch[:, 512:1024]
                                nc.scalar.activation(
                                    out=outb, in_=scratch[:, 512:1024],
                                    func=AFT.Copy,
                                    accum_out=logits_b[:, t:t + 1],
                                )
                    if logits_mode != "ttr":
                        nc.vector.tensor_add(logits_a[:], logits_a[:],
                                             logits_b[:])

                    if stop_after == "logits":
                        nc.sync.dma_start(out=enc[0, b * D:b * D + NT], in_=logits_a[0:1, :])
                        continue

                    if "softmax" in skip:
                        zsum = tiny_pool.tile([1, 1], F32, tag="zsum")
                        nc.vector.memset(zsum[:], 1.0)
                        w_pm = tiny_pool.tile([128, NT], BF16, tag="w_pm")
                        nc.vector.memset(w_pm[:], 0.01)
                        do_tail(b, xh, w_pm, zsum)
                        continue

                    # softmax without the row max: fixed shift (cancels in
                    # enc = sum(W x)/(Q Z)); exp on ACT in [128, NT] space.
                    p_pad = tiny_pool.tile([128, NT + 2], F32, tag="p_pad")
                    zcol = tiny_pool.tile([128, 1], F32, tag="zcol")
                    nc.vector.memset(p_pad[:, 0:1], 0.0)
                    nc.vector.memset(p_pad[:, NT + 1:NT + 2], 0.0)
                    nc.scalar.activation(
                        out=p_pad[:, 1:NT + 1],
                        in_=logits_a[:],
                        func=AFT.Exp,
                        bias=nshift[:],
                        scale=1.0,
                    )
                    nc.vector.tensor_reduce(
                        out=zcol[:], in_=p_pad[:, 1:NT + 1],
                        axis=mybir.AxisListType.X, op=ALU.add,
                    )

                    # Z = sum over partitions of zcol (ones-column matmul)
                    z_ps = psum_pool.tile([1, 1], F32, tag="pr")
                    nc.tensor.matmul(z_ps[:], lhsT=zcol[:], rhs=ones_col[:],
                                     start=True, stop=True)
                    zsum = tiny_pool.tile([1, 1], F32, tag="zsum")
                    nc.scalar.copy(out=zsum[:], in_=z_ps[:])

                    # W = 4-tap window of p: banded matmuls over tiles
                    # 0..14 at once (halo via shifted rhs columns of p_pad);
                    # the last tile separately with s4last (W[M-1] = 0).
                    w_ps = psum_pool.tile([128, NT], F32, tag="w_ps")
                    nc.tensor.matmul(w_ps[:, 0:NT - 1], lhsT=s4[:],
                                     rhs=p_pad[:, 1:NT], start=True, stop=False)
                    nc.tensor.matmul(w_ps[:, 0:NT - 1], lhsT=sprev[:],
                                     rhs=p_pad[:, 0:NT - 1], start=False,
                                     stop=False)
                    nc.tensor.matmul(w_ps[:, 0:NT - 1], lhsT=snext[:],
                                     rhs=p_pad[:, 2:NT + 1], start=False,
                                     stop=True)
                    nc.tensor.matmul(w_ps[:, NT - 1:NT], lhsT=s4last[:],
                                     rhs=p_pad[:, NT:NT + 1], start=True,
                                     stop=False)
                    nc.tensor.matmul(w_ps[:, NT - 1:NT], lhsT=sprev[:],
                                     rhs=p_pad[:, NT - 1:NT], start=False,
                                     stop=True)
                    w_pm = tiny_pool.tile([128, NT], BF16, tag="w_pm")
                    nc.scalar.copy(out=w_pm[:], in_=w_ps[:])

                    do_tail(b, xh, w_pm, zsum)

                if logits_mode == "ttr":
                    for f in pending:
                        f()
                    # NOTE: issuing this from the ACT DGE queue instead was
                    # measured 23us/rep SLOWER on HW (143 vs 120 at
                    # unroll=8) — keep it on the SP sync queue
                    nc.sync.dma_start(out=enc[:], in_=enc_sb[0:1, :])

            def do_tail(b, xh, w_pm, zsum):
                # enc_un[d] = sum_m W[m] x[m, d]   (PE, W cols as weights)
                pe0 = psum_e_pool.tile([1, 512], F32, tag="pe0")
                pe1 = psum_e_pool.tile([1, 512], F32, tag="pe1")
                for t in range(NT):
                    xt = xh[t // HNT][:, (t % HNT) * D:(t % HNT + 1) * D]
                    for dh, pe in enumerate((pe0, pe1)):
                        nc.tensor.matmul(
                            pe[:],
                            lhsT=w_pm[:, t:t + 1],
                            rhs=xt[:, dh * 512:(dh + 1) * 512],
                            start=(t == 0),
                            stop=(t == NT - 1),
                        )

                enc_sb = small_pool.tile([1, BPC * D], F32, tag="enc_sb")
                if stop_after == "mm":
                    nc.scalar.copy(out=enc_sb[:, b * D:b * D + 512], in_=pe0[:])
                    nc.scalar.copy(out=enc_sb[:, b * D + 512:(b + 1) * D],
                                   in_=pe1[:])
                    if b == n_batches - 1:
                        nc.sync.dma_start(out=enc[:], in_=enc_sb[0:1, :])
                    return

                # enc[b] = enc_un / (Q * Z)
                z2 = small_pool.tile([1, 1], F32, tag="z2")
                nc.scalar.mul(out=z2[:], in_=zsum[:], mul=float(Q))
                rz = small_pool.tile([1, 1], F32, tag="rz")
                nc.vector.reciprocal(rz[:], z2[:])
                nc.scalar.activation(
                    out=enc_sb[:, b * D:b * D + 512], in_=pe0[:], func=AFT.Copy,
                    scale=rz[:],
                )
                nc.scalar.activation(
                    out=enc_sb[:, b * D + 512:(b + 1) * D], in_=pe1[:],
                    func=AFT.Copy, scale=rz[:],
                )
                if b == n_batches - 1:
                    nc.sync.dma_start(out=enc[:], in_=enc_sb[0:1, :])

            if reps == 1:
                body()
            elif unroll == 0:
                # flat python unroll: no hardware loop at all (timeline-sim
                # and collective-bearing builds can't use For_i)
                for _ in range(reps):
                    body()
            elif unroll > 1:
                # unrolled loop body: consecutive reps rotate through the
                # tile pools, letting the next rep's P/x DMAs overlap this
                # rep's batch compute (the plain loop edge serializes; a
                # non-loop epilogue body also kills the pipelining, so the
                # rep count rounds UP to a multiple of unroll - reps are
                # idempotent, an extra one only costs time).
                with tc.For_i(0, (reps + unroll - 1) // unroll, 1):
                    for _ in range(unroll):
                        body()
            else:
                with tc.For_i(0, reps, 1):
                    body()

    return nc


def _shard_inputs(embeds_x, embeds_y, P, pshard=False):
    """Build the 8 per-core input maps (host-side layout + bf16 cast)."""
    bf = ml_dtypes.bfloat16
    x = np.asarray(embeds_x, dtype=np.float32)
    y = np.asarray(embeds_y, dtype=np.float32)[:, :, 0]          # [B, CD]
    if pshard:
        ptr_full = P.T.reshape(KT, 128, D).astype(bf)            # [KT, 128, D]
        yk = y.reshape(B, KT, 128).astype(bf)                    # [B, KT, 128]
        in_maps = []
        for c in range(NCORES):
            kt0 = c * KTC
            pt_c = np.zeros((128, KTC * D + PAD), dtype=bf)
            pt_c[:, :KTC * D] = ptr_full[kt0:kt0 + KTC].transpose(
                1, 0, 2).reshape(128, KTC * D)
            ys_c = np.ascontiguousarray(
                yk[:, kt0:kt0 + KTC, :].transpose(2, 1, 0))      # [128, KTC, B]
            sel_c = np.zeros((B, BPC * 128), dtype=bf)
            for b in range(BPC):
                sel_c[c * BPC + b, b * 128:(b + 1) * 128] = 1.0
            sl = slice(c * BPC, (c + 1) * BPC)
            xs_c = np.zeros((128, BPC, NT * D + PAD), dtype=bf)
            xs_c[:, :, :NT * D] = x[sl].reshape(BPC, NT, 128, D).transpose(
                2, 0, 1, 3).reshape(128, BPC, NT * D).astype(bf)
            in_maps.append({"xs": xs_c, "pt": pt_c, "ys": ys_c, "sel": sel_c})
        return in_maps
    # pt[p, k*D + d] = P[d, k*128 + p]
    ptr = np.zeros((128, KT * D + PAD), dtype=bf)
    ptr[:, :KT * D] = P.T.reshape(KT, 128, D).transpose(1, 0, 2).reshape(
        128, KT * D).astype(bf)
    in_maps = []
    for c in range(NCORES):
        sl = slice(c * BPC, (c + 1) * BPC)
        # xs[p, b, t*D + d] = x[b, t*128 + p, d]
        xs_c = np.zeros((128, BPC, NT * D + PAD), dtype=bf)
        xs_c[:, :, :NT * D] = x[sl].reshape(BPC, NT, 128, D).transpose(
            2, 0, 1, 3).reshape(128, BPC, NT * D).astype(bf)
        ys_c = np.ascontiguousarray(
            y[sl].reshape(BPC, KT, 128).transpose(2, 1, 0)
        ).astype(bf)  # [128, KT, BPC]
        in_maps.append({"xs": xs_c, "pt": ptr, "ys": ys_c})
    return in_maps


# canonical build configuration — kernel() and test.py both use this
BUILD_KW = dict(xhalves=True, logits_mode="ttr", n_pool=0, xbufs=3,
                ptbufs=4, unroll=8)


def kernel(embeds_x, embeds_y, P, M):
    assert int(M) == 2048
    nc = build_nc(reps=1, **BUILD_KW)
    split_sync_waits(nc)  # HW-compile only; CoreSim rejects injected NoOps
    in_maps = _shard_inputs(embeds_x, embeds_y, P)
    res = run_bass_kernel_spmd(nc, in_maps, list(range(NCORES)))
    out = np.concatenate(
        [res.results[c]["enc"].reshape(BPC, D) for c in range(NCORES)], axis=0)
    return out.astype(np.float32)



# revision 50
# speedup vs baseline: 1.4653x; 1.0019x over previous
"""Trainium2 Bass kernel for the attention-based encoder.

Computation (per batch b):
    a      = P @ y[b]                                  # [D]
    logits = x[b] @ a                                  # [M]
    p_un   = exp(logits - 16)                          # fixed shift (cancels)
    Z      = sum(p_un)
    W[t]   = p_un[t-1] + p_un[t] + p_un[t+1] + p_un[t+2]  (zero-padded), W[M-1] = 0
    enc[b] = (W @ x[b]) / (Q * Z)                      # [D]

Design (each point validated by HW ablation / TimelineSim analysis):
  * all HBM traffic in bf16 (x 16 MiB + P 10 MiB per core; rel err 3.3e-3
    vs the 2e-2 gate) - halves the DMA floor and runs every matmul at
    bf16 rate instead of multi-pass fp32.
  * host pre-arranges x and P^T so every DMA is one large transfer with
    contiguous 16-80 KiB per partition, and pads the per-partition row
    by 2176 B: the power-of-2 HBM stride otherwise aliases banks
    (187 -> 315 GB/s measured).
  * logits via ONE fused op per m-tile: DVE scalar_tensor_tensor
    (out=(x*1)*a_rep, accum_out=full-1024 dot), ALL 16 tiles on DVE.
    (tensor_tensor_reduce and Pool-side STT both fail walrus codegen
    "ISA wrong length" — only DVE STT works.  Same-process HW sweep:
    n_pool=0/3/5 -> 97.6/113.7/120.9 us per rep — every tile moved to
    the Pool-mul + ACT-accumulate path COSTS ~4.7 us/rep despite the
    cost model predicting it balances; ablating the whole logits stage
    hits the 86.6 us DMA floor, so logits is the only stage above it.
    The n_grp hybrid — 2x-rate DVE group-mul + per-tile ACT 512-half
    accumulates for the first n_grp tiles of each half — also LOSES:
    n_grp=4/5 -> 101.4/109.9 us in a faster era than the 97.6 run.
    Every reduce-offload engine mix measured worse than pure DVE STT.)
  * per-batch split-half pipeline: tiles 0-7 logits -> exp(+Z accum) ->
    W cols 0..6 -> PE tail t0..6 overlap tiles 8-15 logits; W = 4-tap
    window of p via banded matmuls with +-2 halo columns.
  * x halves DMAed in 2 pieces (last batch's h1 in 4) so compute chases
    the bus; last P chunk's DMA split 7+1 k-tiles so phase A trails the
    29-us P transfer by <1 us.
  * a replicated across partitions with a selector matmul; copies ride
    the idle ACT engine, scheduled b0/b1 in phase A and b+2 inside
    batch b (keeps the single 'pr' PSUM bank ring short without the PE
    tail ever gating a batch start).
  * the normalizer chain is deferred: Q folded into the Z ones-matmul
    weights, reciprocal (DVE) + PSUM->enc scales (ACT, dep on PE tail
    stop) of batch b emitted at batch b+2 so the in-order DVE/ACT
    queues never stall between batches.
  * reps>1 builds unroll the For_i body by 8 (HW: 126.9/122.9/120.4 us
    per rep at unroll 2/4/8).  Flat (no-loop) unrolling collapses to
    1.7 ms/rep on HW (instruction-memory thrash) — loops are mandatory.
    Emitting the output DMA from the ACT DGE queue instead of SP was
    23 us/rep slower — keep every DMA on nc.sync.
  * pshard=True phase A (P split over cores + AllReduce) works at
    reps=1 and is equally accurate (3.298e-3), but collectives fail at
    runtime inside a For_i reps-loop, so it stays off.

Sharding: data-parallel over batch, 4 batches per core on 8 cores.
"""

import numpy as np
import ml_dtypes

import concourse.bass as bass
import concourse.mybir as mybir
from concourse.tile import TileContext
from concourse.bass_utils import run_bass_kernel_spmd

# ---------------------------------------------------------------------------
# This container's walrus supports only ONE sync wait per instruction ("Too
# many sync wait commands" at codegen otherwise), while Tile freely attaches
# several.  Post-pass: hoist excess waits onto injected same-engine NoOps
# placed immediately before the over-subscribed instruction.
# ---------------------------------------------------------------------------

_MAX_WAITS = 1


def split_sync_waits(nc: bass.Bass) -> None:
    uid = 0
    for fn in nc.m.functions:
        for blk in fn.blocks:
            new_insts = []
            for inst in blk.instructions:
                si = inst.sync_info
                waits = list(si.on_wait) if si and si.on_wait else []
                if len(waits) > _MAX_WAITS:
                    for w in waits[:-_MAX_WAITS]:
                        uid += 1
                        ev = mybir.InstEventSemaphore(
                            name=f"{inst.name}_hw{uid}",
                            opcode="EventSemaphore",
                            ins=[],
                            outs=[],
                            sync_info=mybir.SyncInfo(on_wait=[w], on_update=[]),
                        )
                        ev.engine = inst.engine
                        new_insts.append(ev)
                    si.on_wait = waits[-_MAX_WAITS:]
                new_insts.append(inst)
            blk.instructions[:] = new_insts

# ---------------------------------------------------------------------------

B, M, D, CD = 32, 2048, 1024, 5120
Q = 2
NCORES = 8
BPC = B // NCORES          # batches per core
NT = M // 128              # m-tiles per batch
KT = CD // 128             # k-tiles of the P contraction
KCH = 8                    # k-tiles per P DMA chunk
NCH = KT // KCH
KTC = KT // NCORES  # k-tiles per core when P is sharded
PAD = 1088                 # bf16 elems of per-partition-row padding: breaks
                           # the power-of-2 HBM stride aliasing (187 -> 315 GB/s)
HNT = NT // 2              # m-tiles per xb half-load
F32 = mybir.dt.float32
BF16 = mybir.dt.bfloat16
ALU = mybir.AluOpType
AFT = mybir.ActivationFunctionType


def build_nc(reps: int = 1, n_batches: int = BPC, do_phase_b: bool = True,
             stop_after: str = "full", skip: tuple = (),
             logits_mode: str = "split", xhalves: bool = True,
             n_dve: int = 16, n_gps: int = 0,
             accum_dump: str = "inplace", fuse_g: int = 4,
             n_actg: int = 0, pshard: bool = False,
             unroll: int = 2, n_pool: int = 0, xpieces: int = 2,
             n_grp: int = 0,
             xbufs: int = 3, ptbufs: int = 3, scrbufs: int = 3) -> bass.Bass:
    nc = bass.Bass(num_devices=NCORES) if pshard else bass.Bass()
    xs = nc.declare_dram_parameter("xs", [128, BPC, NT * D + PAD], BF16, isOutput=False)
    if pshard:
        # P sharded over cores: each core holds KTC k-tiles of P^T, the y
        # slices matching those k-tiles for ALL B batches, and a selector
        # that picks this core's batch rows out of the allreduced aT.
        pt = nc.declare_dram_parameter("pt", [128, KTC * D + PAD], BF16,
                                       isOutput=False)
        ys = nc.declare_dram_parameter("ys", [128, KTC, B], BF16,
                                       isOutput=False)
        sel = nc.declare_dram_parameter("sel", [B, BPC * 128], BF16,
                                        isOutput=False)
        cc_in = nc.dram_tensor("cc_in", [B, D], F32, addr_space="Local")
        cc_out = nc.dram_tensor("cc_out", [B, D], F32, addr_space="Local")
    else:
        pt = nc.declare_dram_parameter("pt", [128, KT * D + PAD], BF16,
                                       isOutput=False)
        ys = nc.declare_dram_parameter("ys", [128, KT, BPC], BF16,
                                       isOutput=False)
    enc = nc.declare_dram_parameter("enc", [1, BPC * D], F32, isOutput=True)

    with TileContext(nc) as tc:
        with (
            tc.tile_pool(name="const", bufs=1) as const_pool,
            tc.tile_pool(name="ysp", bufs=1) as ys_pool,
            tc.tile_pool(name="ptp", bufs=ptbufs) as pt_pool,
            tc.tile_pool(name="xp", bufs=xbufs) as x_pool,
            tc.tile_pool(name="arep", bufs=1) as arep_pool,
            tc.tile_pool(name="small", bufs=1) as small_pool,
            tc.tile_pool(name="tiny", bufs=2) as tiny_pool,
            tc.tile_pool(name="scr", bufs=scrbufs) as scr_pool,
            tc.tile_pool(name="ps", bufs=1, space="PSUM") as psum_pool,
            tc.tile_pool(name="pse", bufs=2, space="PSUM") as psum_e_pool,
        ):
            ones_col = const_pool.tile([128, 1], F32)
            nc.vector.memset(ones_col[:], 1.0)
            # Q-scaled ones: folds the x_bar 1/Q into Z at no cost
            q_col = const_pool.tile([128, 1], F32)
            nc.vector.memset(q_col[:], float(Q))
            nshift = const_pool.tile([128, 1], F32)
            nc.vector.memset(nshift[:], -16.0)
            if pshard:
                ys_sb = const_pool.tile([128, KTC, B], BF16)
                nc.sync.dma_start(out=ys_sb[:], in_=ys[:])
                sel_sb = const_pool.tile([B, BPC * 128], BF16)
                nc.sync.dma_start(out=sel_sb[:], in_=sel[:])
            else:
                ys_sb = const_pool.tile([128, KT, BPC], BF16)
                nc.sync.dma_start(out=ys_sb[:], in_=ys[:])

            # banded matrices for the 4-tap sliding-window sum W = S4 @ p:
            # s4[c, f] = 1 iff f - c in {-2, -1, 0, 1}; corner matrices
            # carry the +-2-element inter-tile halo.
            s4 = const_pool.tile([128, 128], F32)
            nc.gpsimd.memset(s4[:], 0.0)
            for base in (1, 0, -1, -2):
                nc.gpsimd.affine_select(
                    out=s4[:], in_=s4[:], compare_op=ALU.not_equal, fill=1.0,
                    base=base, pattern=[[-1, 128]], channel_multiplier=1,
                )
            sprev = const_pool.tile([128, 128], F32)
            nc.gpsimd.memset(sprev[:], 0.0)
            nc.gpsimd.affine_select(
                out=sprev[:], in_=sprev[:], compare_op=ALU.not_equal, fill=1.0,
                base=-127, pattern=[[-1, 128]], channel_multiplier=1,
            )
            snext = const_pool.tile([128, 128], F32)
            nc.gpsimd.memset(snext[:], 0.0)
            for base in (126, 127):
                nc.gpsimd.affine_select(
                    out=snext[:], in_=snext[:], compare_op=ALU.not_equal, fill=1.0,
                    base=base, pattern=[[-1, 128]], channel_multiplier=1,
                )

            # last-tile variant of s4 with column M-1 zeroed (W[M-1] = 0)
            s4last = const_pool.tile([128, 128], F32)
            nc.gpsimd.memset(s4last[:], 0.0)
            for base in (1, 0, -1, -2):
                nc.gpsimd.affine_select(
                    out=s4last[:], in_=s4last[:], compare_op=ALU.not_equal,
                    fill=1.0, base=base, pattern=[[-1, 128]],
                    channel_multiplier=1,
                )
            nc.gpsimd.affine_select(
                out=s4last[:], in_=s4last[:], compare_op=ALU.not_equal,
                fill=0.0, base=-127, pattern=[[1, 128]], channel_multiplier=0,
            )

            # selector rows: selb[b][k, m] = 1 iff k == b (4 partitions);
            # a_rep[b] = selb[b].T @ aT broadcasts batch b's row of aT to
            # all 128 partitions without an SBUF round-trip.
            selb = []
            for b in range(BPC):
                sb = const_pool.tile([BPC, 128], BF16, name=f"selb{b}")
                nc.gpsimd.memset(sb[:], 0.0)
                nc.gpsimd.affine_select(
                    out=sb[:], in_=sb[:], compare_op=ALU.not_equal, fill=1.0,
                    base=-b, pattern=[[0, 128]], channel_multiplier=1,
                )
                selb.append(sb)

            a_rep = [
                arep_pool.tile([128, D], BF16, tag=f"a_rep{b}", name=f"a_rep{b}")
                for b in range(BPC)
            ]

            def body(_=None):
                if "phase_a" in skip:
                    [nc.vector.memset(ar[:], 0.001) for ar in a_rep]
                    return body_b()

                if pshard:
                    return phase_a_sharded()

                # ---- Phase A: aT[b, d] = sum_k y[b, k] * PT[k, d] ----
                pa0 = psum_pool.tile([BPC, 512], F32, tag="pa0")
                pa1 = psum_pool.tile([BPC, 512], F32, tag="pa1")
                for c in range(NCH):
                    ptc = pt_pool.tile([128, KCH * D], BF16, tag="ptc")
                    if c == NCH - 1:
                        # split the last chunk's DMA so its final k-tile (and
                        # the phase-A tail) lags the P transfer by ~0.8us
                        # instead of a full 2MiB chunk
                        nc.sync.dma_start(
                            out=ptc[:, 0:(KCH - 1) * D],
                            in_=pt[:, c * KCH * D:(c * KCH + KCH - 1) * D],
                        )
                        nc.sync.dma_start(
                            out=ptc[:, (KCH - 1) * D:KCH * D],
                            in_=pt[:, (c * KCH + KCH - 1) * D:(c + 1) * KCH * D],
                        )
                    else:
                        nc.sync.dma_start(
                            out=ptc[:], in_=pt[:, c * KCH * D:(c + 1) * KCH * D]
                        )
                    for u in range(KCH):
                        t = c * KCH + u
                        nc.tensor.matmul(
                            pa0[:], lhsT=ys_sb[:, t, :],
                            rhs=ptc[:, u * D:u * D + 512],
                            start=(t == 0), stop=(t == KT - 1),
                        )
                        nc.tensor.matmul(
                            pa1[:], lhsT=ys_sb[:, t, :],
                            rhs=ptc[:, u * D + 512:(u + 1) * D],
                            start=(t == 0), stop=(t == KT - 1),
                        )
                aT_sb = small_pool.tile([BPC, D], BF16, tag="aT")
                nc.scalar.copy(out=aT_sb[:, 0:512], in_=pa0[:])
                nc.scalar.copy(out=aT_sb[:, 512:1024], in_=pa1[:])

                if logits_mode == "ttr":
                    # replicate a for the first two batches now (copies on
                    # the mostly-idle ACT engine); b+2's replication is
                    # emitted inside batch b so the PSUM 'pr' ring stays
                    # short while batch starts never wait on the PE tail
                    for b in range(min(2, n_batches if do_phase_b else BPC)):
                        replicate_a(b, aT_sb, eng="act")
                else:
                    for b in range(BPC):
                        replicate_a(b, aT_sb, eng="dve")

                if not do_phase_b:
                    for b in range(BPC):
                        if logits_mode == "ttr" and b >= 2:
                            replicate_a(b, aT_sb, eng="act")
                        nc.gpsimd.dma_start(
                            out=enc[0, b * D:b * D + 512],
                            in_=a_rep[b][0:1, 0:512],
                        )
                    return
                return body_b(aT_sb)

            def replicate_a(b, aT_sb, eng):
                for dh in range(2):
                    pr = psum_pool.tile([128, 512], F32, tag="pr")
                    nc.tensor.matmul(
                        pr[:], lhsT=selb[b][:],
                        rhs=aT_sb[:, dh * 512:(dh + 1) * 512],
                        start=True, stop=True,
                    )
                    if eng == "act":
                        nc.scalar.copy(
                            out=a_rep[b][:, dh * 512:(dh + 1) * 512],
                            in_=pr[:],
                        )
                    else:
                        nc.vector.tensor_copy(
                            a_rep[b][:, dh * 512:(dh + 1) * 512], pr[:]
                        )

            def phase_a_sharded():
                # partial aT for ALL batches from this core's P k-tiles,
                # then an 8-core AllReduce combines the partials.
                pa0 = psum_pool.tile([B, 512], F32, tag="pa0")
                pa1 = psum_pool.tile([B, 512], F32, tag="pa1")
                ptc = pt_pool.tile([128, KTC * D], BF16, tag="ptc")
                nc.sync.dma_start(out=ptc[:], in_=pt[:, 0:KTC * D])
                for kt in range(KTC):
                    nc.tensor.matmul(
                        pa0[:], lhsT=ys_sb[:, kt, :],
                        rhs=ptc[:, kt * D:kt * D + 512],
                        start=(kt == 0), stop=(kt == KTC - 1),
                    )
                    nc.tensor.matmul(
                        pa1[:], lhsT=ys_sb[:, kt, :],
                        rhs=ptc[:, kt * D + 512:(kt + 1) * D],
                        start=(kt == 0), stop=(kt == KTC - 1),
                    )
                aT_part = small_pool.tile([B, D], F32, tag="aT_part")
                nc.vector.tensor_copy(aT_part[:, 0:512], pa0[:])
                nc.vector.tensor_copy(aT_part[:, 512:1024], pa1[:])
                nc.sync.dma_start(out=cc_in[:], in_=aT_part[:])
                nc.gpsimd.collective_compute(
                    "AllReduce", ALU.add,
                    replica_groups=[list(range(NCORES))],
                    ins=[cc_in[:]], outs=[cc_out[:]],
                )
                aT_f32 = small_pool.tile([B, D], F32, tag="aT_f32")
                nc.sync.dma_start(out=aT_f32[:], in_=cc_out[:])
                aT_sb = small_pool.tile([B, D], BF16, tag="aT")
                nc.vector.tensor_copy(aT_sb[:], aT_f32[:])

                for b in range(BPC):
                    for dh in range(2):
                        pr = psum_pool.tile([128, 512], F32, tag="pr")
                        nc.tensor.matmul(
                            pr[:], lhsT=sel_sb[:, b * 128:(b + 1) * 128],
                            rhs=aT_sb[:, dh * 512:(dh + 1) * 512],
                            start=True, stop=True,
                        )
                        nc.vector.tensor_copy(
                            a_rep[b][:, dh * 512:(dh + 1) * 512], pr[:]
                        )
                if not do_phase_b:
                    for b in range(BPC):
                        nc.gpsimd.dma_start(
                            out=enc[0, b * D:b * D + 512],
                            in_=a_rep[b][0:1, 0:512],
                        )
                    return
                return body_b()

            def ttr_batch(b, xh, enc_sb):
                # fused logits: one tensor_tensor_reduce (DVE) or
                # scalar_tensor_tensor (Pool) per m-tile — accum_out takes
                # the full 1024-wide dot against a_rep.  The batch is
                # processed in two m-halves so exp/W/tail of half 0 overlap
                # the second half's logits ops (cuts the per-batch drain).
                logits = tiny_pool.tile([128, NT], F32, tag="logits_a")
                logits_b = None
                if n_grp:
                    logits_b = tiny_pool.tile([128, NT], F32, tag="logits_b2")
                p_pad = tiny_pool.tile([128, NT + 2], F32, tag="p_pad")
                w_pm = tiny_pool.tile([128, NT], BF16, tag="w_pm")
                zc = [tiny_pool.tile([128, 1], F32, tag=f"zc{h}",
                                     name=f"zc{h}") for h in range(2)]
                if b < 2:
                    # pad columns are zeroed once per ring buffer (bufs=2);
                    # later batches reuse the already-zeroed slots
                    nc.vector.memset(p_pad[:, 0:1], 0.0)
                    nc.vector.memset(p_pad[:, NT + 1:NT + 2], 0.0)
                if "ttr_tail" not in skip:
                    w_ps = psum_pool.tile([128, NT], F32, tag="w_ps")
                    pe0 = psum_e_pool.tile([1, 512], F32, tag="pe0")
                    pe1 = psum_e_pool.tile([1, 512], F32, tag="pe1")

                for h in range(2):
                    t0, t1 = h * HNT, h * HNT + HNT
                    npool_h = (n_pool + (1 - h)) // 2
                    if "ttr_logits" in skip:
                        # ablation: constant logits, x left unconsumed by
                        # this stage (DMA traffic unchanged)
                        nc.vector.memset(logits[:, t0:t1], 0.005)
                    elif True:
                     if n_grp:
                        # first n_grp tiles of the half via one 2x-rate DVE
                        # group mul + per-tile ACT 512-half accumulates
                        # (~420ns each), offloading reduce work to ACT
                        G = n_grp
                        xt = xh[h][:, 0:G * D]
                        scr = scr_pool.tile([128, G * D], BF16, tag="scr_g")
                        nc.vector.tensor_mul(
                            scr[:].rearrange("p (g d) -> p g d", g=G),
                            xt.rearrange("p (g d) -> p g d", g=G),
                            a_rep[b][:, None, :].broadcast_to([128, G, D]),
                        )
                        for u in range(G):
                            nc.scalar.activation(
                                out=scr[:, u * D:u * D + 512],
                                in_=scr[:, u * D:u * D + 512],
                                func=AFT.Copy,
                                accum_out=logits[:, t0 + u:t0 + u + 1],
                            )
                            nc.scalar.activation(
                                out=scr[:, u * D + 512:(u + 1) * D],
                                in_=scr[:, u * D + 512:(u + 1) * D],
                                func=AFT.Copy,
                                accum_out=logits_b[:, t0 + u:t0 + u + 1],
                            )
                     for t in range(t0 + n_grp, t1):
                        xt = xh[h][:, (t - t0) * D:(t - t0 + 1) * D]
                        if t - t0 < npool_h:
                            # Pool mul + ACT accumulate (Pool cannot encode
                            # the fused STT-with-accum; TTR is unsupported by
                            # this walrus build entirely)
                            scr = scr_pool.tile([128, D], BF16, tag="scr_p")
                            nc.gpsimd.tensor_mul(scr[:], xt, a_rep[b][:])
                            nc.scalar.activation(
                                out=scr[:], in_=scr[:], func=AFT.Copy,
                                accum_out=logits[:, t:t + 1],
                            )
                        else:
                            # fused mul+sum in one DVE op:
                            # out=(x·1)·a, accum_out=Σ_free out
                            scr = scr_pool.tile([128, D], BF16, tag="scr_v")
                            nc.vector.scalar_tensor_tensor(
                                out=scr[:], in0=xt, scalar=1.0,
                                in1=a_rep[b][:], op0=ALU.mult, op1=ALU.mult,
                                accum_out=logits[:, t:t + 1],
                            )

                    if n_grp and "ttr_logits" not in skip:
                        # combine the grouped tiles' two 512-half partials
                        # (on the otherwise-idle Pool engine so neither DVE
                        # nor ACT stalls on the cross-engine join)
                        nc.gpsimd.tensor_add(
                            logits[:, t0:t0 + n_grp],
                            logits[:, t0:t0 + n_grp],
                            logits_b[:, t0:t0 + n_grp],
                        )

                    # softmax numerator for this half (fixed shift cancels);
                    # Z partial lands in zc[h] via the ACT accumulator
                    nc.scalar.activation(
                        out=p_pad[:, 1 + t0:1 + t1], in_=logits[:, t0:t1],
                        func=AFT.Exp, bias=nshift[:], scale=1.0,
                        accum_out=zc[h][:],
                    )

                    # W columns + PE tail for this half (halo: W col c needs
                    # p_pad cols c..c+2, all present once this half's exp is
                    # done)
                    if "ttr_tail" in skip:
                        # ablation: no W / tail matmuls (scales then read
                        # stale PSUM — timing only, output is garbage)
                        continue
                    if h == 0:
                        nc.tensor.matmul(w_ps[:, 0:HNT - 1], lhsT=s4[:],
                                         rhs=p_pad[:, 1:HNT], start=True,
                                         stop=False)
                        nc.tensor.matmul(w_ps[:, 0:HNT - 1], lhsT=sprev[:],
                                         rhs=p_pad[:, 0:HNT - 1], start=False,
                                         stop=False)
                        nc.tensor.matmul(w_ps[:, 0:HNT - 1], lhsT=snext[:],
                                         rhs=p_pad[:, 2:HNT + 1], start=False,
                                         stop=True)
                        nc.scalar.copy(out=w_pm[:, 0:HNT - 1],
                                       in_=w_ps[:, 0:HNT - 1])
                        tail_ts = range(0, HNT - 1)
                    else:
                        nc.tensor.matmul(w_ps[:, HNT - 1:NT - 1], lhsT=s4[:],
                                         rhs=p_pad[:, HNT:NT], start=True,
                                         stop=False)
                        nc.tensor.matmul(w_ps[:, HNT - 1:NT - 1],
                                         lhsT=sprev[:],
                                         rhs=p_pad[:, HNT - 1:NT - 1],
                                         start=False, stop=False)
                        nc.tensor.matmul(w_ps[:, HNT - 1:NT - 1],
                                         lhsT=snext[:],
                                         rhs=p_pad[:, HNT + 1:NT + 1],
                                         start=False, stop=True)
                        nc.tensor.matmul(w_ps[:, NT - 1:NT], lhsT=s4last[:],
                                         rhs=p_pad[:, NT:NT + 1], start=True,
                                         stop=False)
                        nc.tensor.matmul(w_ps[:, NT - 1:NT], lhsT=sprev[:],
                                         rhs=p_pad[:, NT - 1:NT], start=False,
                                         stop=True)
                        nc.scalar.copy(out=w_pm[:, HNT - 1:NT],
                                       in_=w_ps[:, HNT - 1:NT])
                        tail_ts = range(HNT - 1, NT)

                    for t in tail_ts:
                        xt = xh[t // HNT][:, (t % HNT) * D:(t % HNT + 1) * D]
                        for dh, pe in enumerate((pe0, pe1)):
                            nc.tensor.matmul(
                                pe[:], lhsT=w_pm[:, t:t + 1],
                                rhs=xt[:, dh * 512:(dh + 1) * 512],
                                start=(t == 0), stop=(t == NT - 1),
                            )

                # Z' = Q * (sum over partitions and halves): q_col folds the
                # 1/Q into the normalizer for free
                z_ps = psum_pool.tile([1, 1], F32, tag="pr")
                nc.tensor.matmul(z_ps[:], lhsT=zc[0][:], rhs=q_col[:],
                                 start=True, stop=False)
                nc.tensor.matmul(z_ps[:], lhsT=zc[1][:], rhs=q_col[:],
                                 start=False, stop=True)
                z2 = tiny_pool.tile([1, 1], F32, tag="z2")
                nc.scalar.copy(out=z2[:], in_=z_ps[:])

                # the reciprocal (DVE) and the PE-tail-dependent scales (ACT)
                # are deferred two batches so they never stall the in-order
                # engine queues between batches
                def emit_scales():
                    rz = tiny_pool.tile([1, 1], F32, tag="rz")
                    nc.vector.reciprocal(rz[:], z2[:])
                    if "ttr_tail" in skip:
                        nc.gpsimd.memset(enc_sb[:, b * D:(b + 1) * D], 0.1)
                        return
                    nc.scalar.activation(
                        out=enc_sb[:, b * D:b * D + 512], in_=pe0[:],
                        func=AFT.Copy, scale=rz[:],
                    )
                    nc.scalar.activation(
                        out=enc_sb[:, b * D + 512:(b + 1) * D], in_=pe1[:],
                        func=AFT.Copy, scale=rz[:],
                    )
                return emit_scales

            def body_b(aT_sb=None):
                # ---- Phase B: per-batch attention ----
                # n_dve: reduce-halves handed to DVE tensor_reduce instead of
                # ACT accumulate; n_gps: muls offloaded to GpSimd.
                if logits_mode == "ttr":
                    enc_sb = small_pool.tile([1, BPC * D], F32, tag="enc_sb")
                    pending = []
                for b in range(n_batches):
                    if xhalves:
                        # half-batch tiles, DMAed in pieces: compute chases
                        # the bus.  The final half of the final batch lands
                        # in 2-tile pieces so the drain after the last byte
                        # is just a couple of logits ops + the short tail.
                        xh = []
                        for h in range(2):
                            xt_ = x_pool.tile([128, HNT * D], BF16, tag=f"xh{h}")
                            pieces = xpieces
                            if logits_mode == "ttr" and h == 1 and \
                                    b == n_batches - 1:
                                pieces = max(4, xpieces)
                            if logits_mode == "ttr" and h == 0 and b == 0:
                                # 2-tile pieces: b0's first tiles land with
                                # the P tail instead of 6.7us after it, so
                                # the DVE logits chain starts at the a_rep
                                # gate (~37us) rather than the bus gate
                                pieces = max(4, xpieces)
                            per = HNT // pieces
                            for i in range(pieces):
                                nc.sync.dma_start(
                                    out=xt_[:, i * per * D:(i + 1) * per * D],
                                    in_=xs[:, b, (h * HNT + i * per) * D:
                                           (h * HNT + (i + 1) * per) * D],
                                )
                            xh.append(xt_)
                    else:
                        xb = x_pool.tile([128, NT * D], BF16, tag="xh0")
                        nc.sync.dma_start(out=xb[:], in_=xs[:, b, 0:NT * D])
                        xh = [xb[:, 0:HNT * D], xb[:, HNT * D:NT * D]]

                    if logits_mode == "ttr":
                        if aT_sb is not None and b + 2 < n_batches:
                            replicate_a(b + 2, aT_sb, eng="act")
                        if len(pending) >= 2:
                            pending.pop(0)()
                        pending.append(ttr_batch(b, xh, enc_sb))
                        continue

                    # logits[m] = x[m, :] . a  - DVE mul per m-tile, then the
                    # free-dim reduce as two 512-halves on ACT (420ns each vs
                    # 1439ns for a 1024 accum / 1266ns for a DVE reduce).
                    logits_a = tiny_pool.tile([128, NT], F32, tag="logits_a")
                    logits_b = tiny_pool.tile([128, NT], F32, tag="logits_b")
                    if "logits" in skip:
                        nc.vector.memset(logits_a[:], 0.005)
                        nc.vector.memset(logits_b[:], 0.005)
                    elif fuse_g:
                        # G-tile fused DVE ops: one mul + one strided reduce
                        # per group amortizes the ~160-cycle DVE op overhead.
                        G = fuse_g
                        for g in range(NT // G):
                            t0 = g * G
                            h = t0 // HNT
                            xt = xh[h][:, (t0 % HNT) * D:(t0 % HNT + G) * D]
                            scratch = scr_pool.tile([128, G * D], BF16,
                                                    tag="scratch")
                            nc.vector.tensor_mul(
                                scratch[:].rearrange("p (g d) -> p g d", g=G),
                                xt.rearrange("p (g d) -> p g d", g=G),
                                a_rep[b][:, None, :].broadcast_to([128, G, D]),
                            )
                            if g < n_actg:
                                # ACT takes this group's second halves too
                                for u in range(G):
                                    nc.scalar.activation(
                                        out=scratch[:, u * D + 512:(u + 1) * D],
                                        in_=scratch[:, u * D + 512:(u + 1) * D],
                                        func=AFT.Copy,
                                        accum_out=logits_b[:, t0 + u:t0 + u + 1],
                                    )
                            else:
                                sv = scratch[:].rearrange(
                                    "p (g two h) -> p g two h", g=G, two=2)
                                nc.vector.tensor_reduce(
                                    out=logits_b[:, t0:t0 + G],
                                    in_=sv[:, :, 1, :],
                                    axis=mybir.AxisListType.X, op=ALU.add,
                                )
                            for u in range(G):
                                nc.scalar.activation(
                                    out=scratch[:, u * D:u * D + 512],
                                    in_=scratch[:, u * D:u * D + 512],
                                    func=AFT.Copy,
                                    accum_out=logits_a[:, t0 + u:t0 + u + 1],
                                )
                    else:
                        for t in range(NT):
                            xt = xh[t // HNT][:, (t % HNT) * D:(t % HNT + 1) * D]
                            scratch = scr_pool.tile([128, D], BF16, tag="scratch")
                            mul_eng = nc.gpsimd if t < n_gps else nc.vector
                            mul_eng.tensor_mul(scratch[:], xt, a_rep[b][:])
                            if t < n_dve - NT:
                                nc.vector.tensor_reduce(
                                    out=logits_a[:, t:t + 1],
                                    in_=scratch[:, 0:512],
                                    axis=mybir.AxisListType.X, op=ALU.add,
                                )
                            else:
                                if accum_dump == "psum":
                                    dmp = psum_pool.tile([128, 512], F32,
                                                         tag="pa0")
                                    outa = dmp[:]
                                elif accum_dump == "sbuf":
                                    dmp = scr_pool.tile([128, 512], BF16,
                                                        tag="dump")
                                    outa = dmp[:]
                                else:
                                    outa = scratch[:, 0:512]
                                nc.scalar.activation(
                                    out=outa, in_=scratch[:, 0:512],
                                    func=AFT.Copy, accum_out=logits_a[:, t:t + 1],
                                )
                            if t < n_dve:
                                nc.vector.tensor_reduce(
                                    out=logits_b[:, t:t + 1],
                                    in_=scratch[:, 512:1024],
                                    axis=mybir.AxisListType.X, op=ALU.add,
                                )
                            else:
                                if accum_dump == "psum":
                                    dmp = psum_pool.tile([128, 512], F32,
                                                         tag="pa1")
                                    outb = dmp[:]
                                elif accum_dump == "sbuf":
                                    dmp = scr_pool.tile([128, 512], BF16,
                                                        tag="dump")
                                    outb = dmp[:]
                                else:
                                    outb = scratch[:, 512:1024]
                                nc.scalar.activation(
                                    out=outb, in_=scratch[:, 512:1024],
                                    func=AFT.Copy,
                                    accum_out=logits_b[:, t:t + 1],
                                )
                    if logits_mode != "ttr":
                        nc.vector.tensor_add(logits_a[:], logits_a[:],
                                             logits_b[:])

                    if stop_after == "logits":
                        nc.sync.dma_start(out=enc[0, b * D:b * D + NT], in_=logits_a[0:1, :])
                        continue

                    if "softmax" in skip:
                        zsum = tiny_pool.tile([1, 1], F32, tag="zsum")
                        nc.vector.memset(zsum[:], 1.0)
                        w_pm = tiny_pool.tile([128, NT], BF16, tag="w_pm")
                        nc.vector.memset(w_pm[:], 0.01)
                        do_tail(b, xh, w_pm, zsum)
                        continue

                    # softmax without the row max: fixed shift (cancels in
                    # enc = sum(W x)/(Q Z)); exp on ACT in [128, NT] space.
                    p_pad = tiny_pool.tile([128, NT + 2], F32, tag="p_pad")
                    zcol = tiny_pool.tile([128, 1], F32, tag="zcol")
                    nc.vector.memset(p_pad[:, 0:1], 0.0)
                    nc.vector.memset(p_pad[:, NT + 1:NT + 2], 0.0)
                    nc.scalar.activation(
                        out=p_pad[:, 1:NT + 1],
                        in_=logits_a[:],
                        func=AFT.Exp,
                        bias=nshift[:],
                        scale=1.0,
                    )
                    nc.vector.tensor_reduce(
                        out=zcol[:], in_=p_pad[:, 1:NT + 1],
                        axis=mybir.AxisListType.X, op=ALU.add,
                    )

                    # Z = sum over partitions of zcol (ones-column matmul)
                    z_ps = psum_pool.tile([1, 1], F32, tag="pr")
                    nc.tensor.matmul(z_ps[:], lhsT=zcol[:], rhs=ones_col[:],
                                     start=True, stop=True)
                    zsum = tiny_pool.tile([1, 1], F32, tag="zsum")
                    nc.scalar.copy(out=zsum[:], in_=z_ps[:])

                    # W = 4-tap window of p: banded matmuls over tiles
                    # 0..14 at once (halo via shifted rhs columns of p_pad);
                    # the last tile separately with s4last (W[M-1] = 0).
                    w_ps = psum_pool.tile([128, NT], F32, tag="w_ps")
                    nc.tensor.matmul(w_ps[:, 0:NT - 1], lhsT=s4[:],
                                     rhs=p_pad[:, 1:NT], start=True, stop=False)
                    nc.tensor.matmul(w_ps[:, 0:NT - 1], lhsT=sprev[:],
                                     rhs=p_pad[:, 0:NT - 1], start=False,
                                     stop=False)
                    nc.tensor.matmul(w_ps[:, 0:NT - 1], lhsT=snext[:],
                                     rhs=p_pad[:, 2:NT + 1], start=False,
                                     stop=True)
                    nc.tensor.matmul(w_ps[:, NT - 1:NT], lhsT=s4last[:],
                                     rhs=p_pad[:, NT:NT + 1], start=True,
                                     stop=False)
                    nc.tensor.matmul(w_ps[:, NT - 1:NT], lhsT=sprev[:],
                                     rhs=p_pad[:, NT - 1:NT], start=False,
                                     stop=True)
                    w_pm = tiny_pool.tile([128, NT], BF16, tag="w_pm")
                    nc.scalar.copy(out=w_pm[:], in_=w_ps[:])

                    do_tail(b, xh, w_pm, zsum)

                if logits_mode == "ttr":
                    for f in pending:
                        f()
                    # NOTE: issuing this from the ACT DGE queue instead was
                    # measured 23us/rep SLOWER on HW (143 vs 120 at
                    # unroll=8) — keep it on the SP sync queue
                    nc.sync.dma_start(out=enc[:], in_=enc_sb[0:1, :])

            def do_tail(b, xh, w_pm, zsum):
                # enc_un[d] = sum_m W[m] x[m, d]   (PE, W cols as weights)
                pe0 = psum_e_pool.tile([1, 512], F32, tag="pe0")
                pe1 = psum_e_pool.tile([1, 512], F32, tag="pe1")
                for t in range(NT):
                    xt = xh[t // HNT][:, (t % HNT) * D:(t % HNT + 1) * D]
                    for dh, pe in enumerate((pe0, pe1)):
                        nc.tensor.matmul(
                            pe[:],
                            lhsT=w_pm[:, t:t + 1],
                            rhs=xt[:, dh * 512:(dh + 1) * 512],
                            start=(t == 0),
                            stop=(t == NT - 1),
                        )

                enc_sb = small_pool.tile([1, BPC * D], F32, tag="enc_sb")
                if stop_after == "mm":
                    nc.scalar.copy(out=enc_sb[:, b * D:b * D + 512], in_=pe0[:])
                    nc.scalar.copy(out=enc_sb[:, b * D + 512:(b + 1) * D],
                                   in_=pe1[:])
                    if b == n_batches - 1:
                        nc.sync.dma_start(out=enc[:], in_=enc_sb[0:1, :])
                    return

                # enc[b] = enc_un / (Q * Z)
                z2 = small_pool.tile([1, 1], F32, tag="z2")
                nc.scalar.mul(out=z2[:], in_=zsum[:], mul=float(Q))
                rz = small_pool.tile([1, 1], F32, tag="rz")
                nc.vector.reciprocal(rz[:], z2[:])
                nc.scalar.activation(
                    out=enc_sb[:, b * D:b * D + 512], in_=pe0[:], func=AFT.Copy,
                    scale=rz[:],
                )
                nc.scalar.activation(
                    out=enc_sb[:, b * D + 512:(b + 1) * D], in_=pe1[:],
                    func=AFT.Copy, scale=rz[:],
                )
                if b == n_batches - 1:
                    nc.sync.dma_start(out=enc[:], in_=enc_sb[0:1, :])

            if reps == 1:
                body()
            elif unroll == 0:
                # flat python unroll: no hardware loop at all (timeline-sim
                # and collective-bearing builds can't use For_i)
                for _ in range(reps):
                    body()
            elif unroll > 1:
                # unrolled loop body: consecutive reps rotate through the
                # tile pools, letting the next rep's P/x DMAs overlap this
                # rep's batch compute (the plain loop edge serializes; a
                # non-loop epilogue body also kills the pipelining, so the
                # rep count rounds UP to a multiple of unroll - reps are
                # idempotent, an extra one only costs time).
                with tc.For_i(0, (reps + unroll - 1) // unroll, 1):
                    for _ in range(unroll):
                        body()
            else:
                with tc.For_i(0, reps, 1):
                    body()

    return nc


def _shard_inputs(embeds_x, embeds_y, P, pshard=False):
    """Build the 8 per-core input maps (host-side layout + bf16 cast)."""
    bf = ml_dtypes.bfloat16
    x = np.asarray(embeds_x, dtype=np.float32)
    y = np.asarray(embeds_y, dtype=np.float32)[:, :, 0]          # [B, CD]
    if pshard:
        ptr_full = P.T.reshape(KT, 128, D).astype(bf)            # [KT, 128, D]
        yk = y.reshape(B, KT, 128).astype(bf)                    # [B, KT, 128]
        in_maps = []
        for c in range(NCORES):
            kt0 = c * KTC
            pt_c = np.zeros((128, KTC * D + PAD), dtype=bf)
            pt_c[:, :KTC * D] = ptr_full[kt0:kt0 + KTC].transpose(
                1, 0, 2).reshape(128, KTC * D)
            ys_c = np.ascontiguousarray(
                yk[:, kt0:kt0 + KTC, :].transpose(2, 1, 0))      # [128, KTC, B]
            sel_c = np.zeros((B, BPC * 128), dtype=bf)
            for b in range(BPC):
                sel_c[c * BPC + b, b * 128:(b + 1) * 128] = 1.0
            sl = slice(c * BPC, (c + 1) * BPC)
            xs_c = np.zeros((128, BPC, NT * D + PAD), dtype=bf)
            xs_c[:, :, :NT * D] = x[sl].reshape(BPC, NT, 128, D).transpose(
                2, 0, 1, 3).reshape(128, BPC, NT * D).astype(bf)
            in_maps.append({"xs": xs_c, "pt": pt_c, "ys": ys_c, "sel": sel_c})
        return in_maps
    # pt[p, k*D + d] = P[d, k*128 + p]
    ptr = np.zeros((128, KT * D + PAD), dtype=bf)
    ptr[:, :KT * D] = P.T.reshape(KT, 128, D).transpose(1, 0, 2).reshape(
        128, KT * D).astype(bf)
    in_maps = []
    for c in range(NCORES):
        sl = slice(c * BPC, (c + 1) * BPC)
        # xs[p, b, t*D + d] = x[b, t*128 + p, d]
        xs_c = np.zeros((128, BPC, NT * D + PAD), dtype=bf)
        xs_c[:, :, :NT * D] = x[sl].reshape(BPC, NT, 128, D).transpose(
            2, 0, 1, 3).reshape(128, BPC, NT * D).astype(bf)
        ys_c = np.ascontiguousarray(
            y[sl].reshape(BPC, KT, 128).transpose(2, 1, 0)
        ).astype(bf)  # [128, KT, BPC]
        in_maps.append({"xs": xs_c, "pt": ptr, "ys": ys_c})
    return in_maps


# canonical build configuration — kernel() and test.py both use this
BUILD_KW = dict(xhalves=True, logits_mode="ttr", n_pool=0, xbufs=3,
                ptbufs=4, unroll=8)


def kernel(embeds_x, embeds_y, P, M):
    assert int(M) == 2048
    nc = build_nc(reps=1, **BUILD_KW)
    split_sync_waits(nc)  # HW-compile only; CoreSim rejects injected NoOps
    in_maps = _shard_inputs(embeds_x, embeds_y, P)
    res = run_bass_kernel_spmd(nc, in_maps, list(range(NCORES)))
    out = np.concatenate(
        [res.results[c]["enc"].reshape(BPC, D) for c in range(NCORES)], axis=0)
    return out.astype(np.float32)

